# revision 1
# baseline (speedup 1.0000x reference)
"""Multi-head attention (B=2, S=2048, D=1024, H=16, dk=64) on 8 TRN2 cores.

Sharding: core c -> (batch b = c//4, head-group g = c%4 of 4 heads).
Each core computes q/k/v projections for its 4 heads, full attention for
those heads, and a partial output projection (rows g*256:(g+1)*256 of Wo).
Host pre-transposes/casts inputs to bf16 and sums the partial outputs.

Device layout (per core, all matmul operands bf16, accumulation f32):
  xqT/xkT/xvT [1024, 2048]   (d on partitions -> contraction-ready)
  qT, kT      [256, 2048]    (head-dim on partitions; pair tiles [128, S])
  v_aug       [2048, 4*65]   (per head: [v_h | ones]; ones col => softmax denom)
  scoresT     [j, i] in PSUM; exp on ScalarE -> probsT bf16 (no max-subtract:
              scores ~ N(0,1) after 1/8 scaling, exp bounded ~e^6)
  PV:         attnT_unnorm[e, i] = sum_j v_aug[j, e] * probsT[j, i]
              (row 64 = softmax denominator), normalize via reciprocal +
              K=1 broadcast matmul, store attnT [64, S] per head
  out-projT:  outT[n, s] = sum_{h,e} wo[h,e,n] * attnT_h[e, s]  (K=64 x4)
Host: out[b] = sum_g outT_partial.T + (bv @ Wo + bo).
"""

import os

import numpy as np
import ml_dtypes

BF16 = ml_dtypes.bfloat16

B, S, D = 2, 2048, 1024
H, DK = 16, 64
P = 128
GROUPS = 4          # head groups (one per core within a batch)
HPG = 4             # heads per group
GD = HPG * DK       # 256, group width
KC = D // P         # 8 contraction chunks
ST = S // P         # 16 s-tiles / j-tiles
NCORES = 8
FP8_PV = False      # fp8 PV measured 3.7e-2 rel err (e4m3 noise) - keep bf16
DEBUG_DUMP = False  # extra outputs: per-head attnT and denominators

_cached = {}


def _build_bass():
    import concourse.bass as bass
    import concourse.tile as tile
    from concourse.bacc import Bacc
    from concourse import mybir
    from contextlib import ExitStack

    f32 = mybir.dt.float32
    bf16 = mybir.dt.bfloat16
    Act = mybir.ActivationFunctionType

    nc = Bacc()

    xqT = nc.dram_tensor("xqT", [D, S], bf16, kind="ExternalInput")
    xkT = nc.dram_tensor("xkT", [D, S], bf16, kind="ExternalInput")
    xvT = nc.dram_tensor("xvT", [D, S], bf16, kind="ExternalInput")
    wq = nc.dram_tensor("wq", [D, GD], bf16, kind="ExternalInput")
    wk = nc.dram_tensor("wk", [D, GD], bf16, kind="ExternalInput")
    wv = nc.dram_tensor("wv", [D, GD], bf16, kind="ExternalInput")
    wo = nc.dram_tensor("wo", [GD, D], bf16, kind="ExternalInput")
    bq = nc.dram_tensor("bq", [GD, 1], f32, kind="ExternalInput")
    bk = nc.dram_tensor("bk", [GD, 1], f32, kind="ExternalInput")
    out = nc.dram_tensor("out", [S, D], f32, kind="ExternalOutput")

    with tile.TileContext(nc) as tc, ExitStack() as ctx:
        singles = ctx.enter_context(tc.tile_pool(name="singles", bufs=1))
        probs_pool = ctx.enter_context(tc.tile_pool(name="probs", bufs=3))
        small = ctx.enter_context(tc.tile_pool(name="small", bufs=8))
        outs_pool = ctx.enter_context(tc.tile_pool(name="outs", bufs=8))
        psum = ctx.enter_context(tc.tile_pool(name="psum", bufs=1, space="PSUM"))

        # ---- persistent SBUF ----
        wq_sb = singles.tile([P, KC, GD], bf16)
        wk_sb = singles.tile([P, KC, GD], bf16)
        wv_sb = singles.tile([P, KC, GD], bf16)
        wo_sb = singles.tile([P, 2, D], bf16)
        bq_sb = singles.tile([P, 2, 1], f32)
        bk_sb = singles.tile([P, 2, 1], f32)
        nc.sync.dma_start(out=wq_sb, in_=wq.rearrange("(c p) m -> p c m", p=P))
        nc.sync.dma_start(out=wk_sb, in_=wk.rearrange("(c p) m -> p c m", p=P))
        nc.sync.dma_start(out=wv_sb, in_=wv.rearrange("(c p) m -> p c m", p=P))
        nc.sync.dma_start(out=wo_sb, in_=wo.rearrange("(c p) n -> p c n", p=P))
        nc.sync.dma_start(out=bq_sb, in_=bq.rearrange("(t p) o -> p t o", p=P))
        nc.sync.dma_start(out=bk_sb, in_=bk.rearrange("(t p) o -> p t o", p=P))

        xq_sb = singles.tile([P, KC, S], bf16)
        xk_sb = singles.tile([P, KC, S], bf16)
        xv_sb = singles.tile([P, KC, S], bf16)
        # tensor-by-tensor so q-proj can start after the first xq chunk
        # and PE chases the DMA stream instead of waiting on all three
        for k in range(KC):
            nc.sync.dma_start(out=xq_sb[:, k, :], in_=xqT[k * P:(k + 1) * P, :])
        for k in range(KC):
            nc.sync.dma_start(out=xk_sb[:, k, :], in_=xkT[k * P:(k + 1) * P, :])
        for k in range(KC):
            nc.sync.dma_start(out=xv_sb[:, k, :], in_=xvT[k * P:(k + 1) * P, :])

        qT_sb = [singles.tile([P, S], bf16, name=f"qT{t}") for t in range(2)]
        kT_sb = [singles.tile([P, S], bf16, name=f"kT{t}") for t in range(2)]
        # attnT per head pair [128 hd, S]: even head at partitions 0:64
        # (written directly by DVE), odd head at 64:128 (DVE writes a base-0
        # staging tile, then SBUF->SBUF DMA relocates partitions - engines
        # are lane-locked but DMA is not). Enables K=128 out-projection.
        att_pair = [singles.tile([P, S], bf16, name=f"attp{p}")
                    for p in range(2)]
        att_odd = [singles.tile([DK, S], bf16, name=f"atto{p}")
                   for p in range(2)]

        ones_sb = singles.tile([65, DK], f32)
        nc.vector.memset(ones_sb[64:65, :], 1.0)

        CP = ST // 2
        if FP8_PV:
            fp8 = mybir.dt.float8e4
            # [j-in-chunk, chunk-pair, chunk-in-pair, head, 64 v cols + 1 one + pad]
            v_sb = singles.tile([P, CP, 2, HPG, 80], fp8)
            nc.vector.memset(v_sb[:, :, :, :, 64:65], 1.0)
            v4 = None
            # exp(s/8 - 3): keeps exp within IEEE e4m3 range (max finite 240;
            # max observed score ~7.7 -> e^4.7 ~ 110). Softmax shift-invariant.
            exp_bias = singles.tile([P, 1], f32)
            nc.vector.memset(exp_bias, -3.0)
        else:
            v_sb = singles.tile([P, ST, HPG * 65], bf16)
            # ones columns of v_aug (col 64 of each per-head [64|1] block)
            v4 = v_sb.rearrange("p s (h c) -> p s h c", c=65)
            nc.vector.memset(v4[:, :, :, 64:65], 1.0)

        # ---- phase A: projections ----
        def qk_proj(x_sb, w_sb, b_sb, dst, t):
            pq = [psum.tile([P, 1024], mybir.dt.float32, tag="sc", bufs=2,
                            name=f"pq{t}{half}") for half in range(2)]
            for k in range(KC):
                for half in range(2):
                    for sq in range(2):
                        nc.tensor.matmul(
                            out=pq[half][:, sq * 512:(sq + 1) * 512],
                            lhsT=w_sb[:, k, t * P:(t + 1) * P],
                            rhs=x_sb[:, k, half * 1024 + sq * 512:
                                     half * 1024 + (sq + 1) * 512],
                            start=(k == 0), stop=(k == KC - 1))
            for half in range(2):
                nc.vector.tensor_scalar_add(
                    out=dst[:, half * 1024:(half + 1) * 1024],
                    in0=pq[half], scalar1=b_sb[:, t, :])

        def v_proj():
            for st in range(ST):
                pvv = psum.tile([P, GD], mybir.dt.float32, tag="pv", bufs=4, name="pvv")
                for k in range(KC):
                    nc.tensor.matmul(
                        out=pvv,
                        lhsT=xv_sb[:, k, st * P:(st + 1) * P],
                        rhs=wv_sb[:, k, :],
                        start=(k == 0), stop=(k == KC - 1))
                if FP8_PV:
                    dst = v_sb[:, st // 2, st % 2, :, 0:64]
                else:
                    dst = v4[:, st, :, 0:64]
                src = pvv.rearrange("p (h c) -> p h c", c=64)
                nc.vector.tensor_copy(out=dst, in_=src)

        # ---- phase B: attention for one head pair, one i-half ----
        # `pending` = previous iteration's normalize emitter; it is emitted
        # after this iteration's first two j-tiles so ACT/PE stay fed across
        # the (pair, ih) boundary. Returns this iteration's normalize.
        def attention(pair, ih, pending=None):
            pv = [[psum.tile([65, 512], mybir.dt.float32, tag="pv", bufs=4,
                             name=f"pv{pair}{ih}{hp}{iq}")
                   for iq in range(2)] for hp in range(2)]
            if FP8_PV:
                fp8 = mybir.dt.float8e4
                for cp in range(CP):
                    pr = [probs_pool.tile([P, 2, 1024], fp8, tag="probs",
                                          name=f"pr{hp}") for hp in range(2)]
                    for d in range(2):
                        jt = 2 * cp + d
                        sc = [psum.tile([P, 1024], mybir.dt.float32, tag="sc",
                                        bufs=2, name=f"sc{hp}")
                              for hp in range(2)]
                        for iq in range(2):
                            for hp in range(2):
                                nc.tensor.matmul(
                                    out=sc[hp][:, iq * 512:(iq + 1) * 512],
                                    lhsT=kT_sb[pair][hp * 64:(hp + 1) * 64,
                                                     jt * P:(jt + 1) * P],
                                    rhs=qT_sb[pair][hp * 64:(hp + 1) * 64,
                                                    ih * 1024 + iq * 512:
                                                    ih * 1024 + (iq + 1) * 512],
                                    start=True, stop=True)
                        for hp in range(2):
                            # exp(s/8 - 2): global shift keeps exp within
                            # e4m3 range (softmax is shift-invariant)
                            nc.scalar.activation(out=pr[hp][:, d, :],
                                                 in_=sc[hp], func=Act.Exp,
                                                 scale=0.125, bias=exp_bias)
                    for hp in range(2):
                        h = 2 * pair + hp
                        for iq in range(2):
                            nc.tensor.matmul(
                                out=pv[hp][iq][:, :],
                                lhsT=v_sb[:, cp, :, h, 0:65],
                                rhs=pr[hp][:, :, iq * 512:(iq + 1) * 512],
                                perf_mode=mybir.MatmulPerfMode.DoubleRow,
                                start=(cp == 0), stop=(cp == CP - 1))
                    if cp == 1 and pending is not None:
                        pending()
            else:
                for jt in range(ST):
                    sc = [psum.tile([P, 1024], mybir.dt.float32, tag="sc",
                                    bufs=2, name=f"sc{hp}") for hp in range(2)]
                    for iq in range(2):
                        for hp in range(2):
                            nc.tensor.matmul(
                                out=sc[hp][:, iq * 512:(iq + 1) * 512],
                                lhsT=kT_sb[pair][hp * 64:(hp + 1) * 64,
                                                 jt * P:(jt + 1) * P],
                                rhs=qT_sb[pair][hp * 64:(hp + 1) * 64,
                                                ih * 1024 + iq * 512:
                                                ih * 1024 + (iq + 1) * 512],
                                start=True, stop=True)
                    for hp in range(2):
                        probs = probs_pool.tile([P, 1024], bf16, tag="probs",
                                                name="probs")
                        nc.scalar.activation(out=probs, in_=sc[hp],
                                             func=Act.Exp, scale=0.125)
                        h65 = (2 * pair + hp) * 65
                        for iq in range(2):
                            nc.tensor.matmul(
                                out=pv[hp][iq][:, :],
                                lhsT=v_sb[:, jt, h65:h65 + 65],
                                rhs=probs[:, iq * 512:(iq + 1) * 512],
                                start=(jt == 0), stop=(jt == ST - 1))
                    if jt == 1 and pending is not None:
                        pending()

            def normalize():
                for hp in range(2):
                    for iq in range(2):
                        r = small.tile([65, 512], mybir.dt.float32, tag="r",
                                       name="r")
                        nc.vector.reciprocal(out=r[64:65, :],
                                             in_=pv[hp][iq][64:65, :])
                        bc = psum.tile([64, 512], mybir.dt.float32, tag="pv",
                                       bufs=4, name="bc")
                        nc.tensor.matmul(out=bc, lhsT=ones_sb[64:65, :],
                                         rhs=r[64:65, :], start=True,
                                         stop=True)
                        pvs = small.tile([64, 512], mybir.dt.float32,
                                         tag="pvs", name="pvs")
                        nc.vector.tensor_copy(out=pvs, in_=pv[hp][iq][0:64, :])
                        col = ih * 1024 + iq * 512
                        if hp == 0:
                            nc.vector.tensor_mul(
                                out=att_pair[pair][0:64, col:col + 512],
                                in0=pvs, in1=bc)
                        else:
                            nc.vector.tensor_mul(
                                out=att_odd[pair][:, col:col + 512],
                                in0=pvs, in1=bc)
                            nc.sync.dma_start(
                                out=att_pair[pair][64:128, col:col + 512],
                                in_=att_odd[pair][:, col:col + 512])

            return normalize

        def out_proj():
            # out[s, n] = sum_c att_pair[c].T @ wo_chunk[c]  (K=128 per chunk)
            for st in range(ST):
                po = [psum.tile([P, 512], mybir.dt.float32, tag="pv", bufs=4,
                                name=f"po{nb}") for nb in range(2)]
                for c in range(2):
                    for nb in range(2):
                        nc.tensor.matmul(
                            out=po[nb],
                            lhsT=att_pair[c][:, st * P:(st + 1) * P],
                            rhs=wo_sb[:, c, nb * 512:(nb + 1) * 512],
                            start=(c == 0), stop=(c == 1))
                for nb in range(2):
                    osb = outs_pool.tile([P, 512], mybir.dt.float32,
                                         tag="osb", name="osb")
                    if nb % 2 == 0:
                        nc.vector.tensor_copy(out=osb, in_=po[nb])
                    else:
                        nc.scalar.copy(out=osb, in_=po[nb])
                    nc.sync.dma_start(
                        out=out[st * P:(st + 1) * P,
                                nb * 512:(nb + 1) * 512],
                        in_=osb)

        qk_proj(xq_sb, wq_sb, bq_sb, qT_sb[0], 0)
        qk_proj(xk_sb, wk_sb, bk_sb, kT_sb[0], 0)
        v_proj()
        qk_proj(xq_sb, wq_sb, bq_sb, qT_sb[1], 1)
        qk_proj(xk_sb, wk_sb, bk_sb, kT_sb[1], 1)
        pending = None
        for pair in range(2):
            for ih in range(2):
                pending = attention(pair, ih, pending)
        pending()
        out_proj()

    nc.finalize()
    return nc


def kernel(Q, K, V, Wq, bq, Wk, bk, Wv, bv, Wo, bo):
    from concourse.bass_utils import run_bass_kernel_spmd

    f32 = np.float32
    Q = np.asarray(Q, f32)
    K = np.asarray(K, f32)
    V = np.asarray(V, f32)
    Wq = np.asarray(Wq, f32)
    Wk = np.asarray(Wk, f32)
    Wv = np.asarray(Wv, f32)
    Wo = np.asarray(Wo, f32)
    bq = np.asarray(bq, f32)
    bk = np.asarray(bk, f32)
    bv = np.asarray(bv, f32)
    bo = np.asarray(bo, f32)

    xT = {}
    for b in range(B):
        xT[('q', b)] = np.ascontiguousarray(Q[b].T).astype(BF16)
        xT[('k', b)] = np.ascontiguousarray(K[b].T).astype(BF16)
        xT[('v', b)] = np.ascontiguousarray(V[b].T).astype(BF16)

    in_maps = []
    for c in range(NCORES):
        b, g = c // GROUPS, c % GROUPS
        sl = slice(g * GD, (g + 1) * GD)
        in_maps.append({
            "xqT": xT[('q', b)],
            "xkT": xT[('k', b)],
            "xvT": xT[('v', b)],
            "wq": np.ascontiguousarray(Wq[:, sl]).astype(BF16),
            "wk": np.ascontiguousarray(Wk[:, sl]).astype(BF16),
            "wv": np.ascontiguousarray(Wv[:, sl]).astype(BF16),
            "wo": np.ascontiguousarray(Wo[sl, :]).astype(BF16),
            "bq": np.ascontiguousarray(bq[sl].reshape(GD, 1)),
            "bk": np.ascontiguousarray(bk[sl].reshape(GD, 1)),
        })

    if "nc" not in _cached:
        _cached["nc"] = _build_bass()
    nc = _cached["nc"]

    try:
        res = run_bass_kernel_spmd(nc, in_maps, core_ids=list(range(NCORES)))
    except ModuleNotFoundError:
        # BASS_TRACE set but the axon ntff hook isn't shipped in this
        # container - retry untraced
        os.environ["BASS_NEVER_TRACE"] = "1"
        res = run_bass_kernel_spmd(nc, in_maps, core_ids=list(range(NCORES)))
    if res.exec_time_ns is not None:
        print(f"HW exec time: {res.exec_time_ns} ns")

    bo_eff = (bv @ Wo + bo).astype(f32)
    out = np.zeros((B, S, D), f32)
    for c in range(NCORES):
        b = c // GROUPS
        out[b] += res.results[c]["out"]
    out += bo_eff
    return out



# revision 52
# speedup vs baseline: 1.3319x; 1.3319x over previous
"""Multi-head attention (B=2, S=2048, D=1024, H=16, dk=64) on 8 TRN2 cores.

Sharding: core c -> (batch b = c//4, head-group g = c%4 of 4 heads).
Each core computes q/k/v projections for its 4 heads, full attention for
those heads, and a partial output projection (rows g*256:(g+1)*256 of Wo).
Host pre-transposes/casts inputs to bf16 and sums the partial outputs.

Structure (per core, matmul operands bf16, accumulation f32):
  xqT/xkT/xvT [1024, 2048]  (d on partitions -> contraction-ready)
  qT, kT      [128, 2048]   per head pair (head-dim on partitions)
  v4          [128 j, 16 jt, 4h x 65]  (per head: [v_h | ones]; ones col
              makes PV's 65th output column the softmax denominator)
  scoresT     sc[j, i-block] in PSUM (2 banks); exp on ScalarE -> probs
              [j, i] bf16 (no max-subtract: scores/8 ~ N(0,1))
  PV          pv[i, e] += probs[:, it-slice].T @ v_aug  (N=65 matmuls,
              accumulated over 16 j-tiles; col 64 = denominator)
  normalize   DVE: r = 1/den; attn = pv * r (broadcast tensor_tensor over
              runs of units)
  transpose   DMA xbar: attn [i, e] -> att2 [e2=(hp,e), pair, i]
  out-projT   out[s, n] = sum_c att2[:, c, s-slice].T @ wo[c]  (K=256)

The jt loop is software-pipelined so ScalarE exp calls run back-to-back
(the bottleneck engine). All other TensorE work (later head-pair q/k
projections, the V projection, the output projection) is chopped into
<=430ns chunks by generators and drip-fed into TensorE's slack through a
single rotating PSUM "aux" bank. Each generator carries an
earliest-unit gate matched to the input-DMA arrival order, and producers
are always EMITTED before their consumers (Tile's dependency tracking is
emission-ordered). Host: out[b] = sum_g outT_partial + (bv @ Wo + bo).
"""

import os
from collections import deque

import numpy as np
import ml_dtypes

BF16 = ml_dtypes.bfloat16

B, S, D = 2, 2048, 1024
H, DK = 16, 64
P = 128
GROUPS = 4          # head groups (one per core within a batch)
HPG = 4             # heads per group
GD = HPG * DK       # 256, group width
KC = D // P         # 8 contraction chunks
ST = S // P         # 16 s-tiles / j-tiles
NCORES = 8
DEBUG = False
SERIAL = False

_cached = {}


def _build_bass():
    import concourse.bass as bass
    import concourse.tile as tile
    from concourse.bacc import Bacc
    from concourse import mybir
    from contextlib import ExitStack

    f32 = mybir.dt.float32
    bf16 = mybir.dt.bfloat16
    Act = mybir.ActivationFunctionType
    Alu = mybir.AluOpType

    nc = Bacc()

    xqT = nc.dram_tensor("xqT", [D, S], bf16, kind="ExternalInput")
    xkT = nc.dram_tensor("xkT", [D, S], bf16, kind="ExternalInput")
    xvT = nc.dram_tensor("xvT", [D, S], bf16, kind="ExternalInput")
    # wq | wk | wv concatenated column-wise
    wqkv = nc.dram_tensor("wqkv", [D, 3 * GD], bf16, kind="ExternalInput")
    wo = nc.dram_tensor("wo", [GD, D], bf16, kind="ExternalInput")
    bq = nc.dram_tensor("bq", [GD, 1], f32, kind="ExternalInput")
    bk = nc.dram_tensor("bk", [GD, 1], f32, kind="ExternalInput")
    ident = nc.dram_tensor("ident", [P, P], bf16, kind="ExternalInput")
    out = nc.dram_tensor("out", [S, D], bf16, kind="ExternalOutput")

    with tile.TileContext(nc) as tc, ExitStack() as ctx:
        singles = ctx.enter_context(tc.tile_pool(name="singles", bufs=1))
        probs_pool = ctx.enter_context(tc.tile_pool(name="probs", bufs=4))
        norm_pool = ctx.enter_context(tc.tile_pool(name="normp", bufs=2))
        small = ctx.enter_context(tc.tile_pool(name="small", bufs=2))
        outs_pool = ctx.enter_context(tc.tile_pool(name="outs", bufs=4))
        psum = ctx.enter_context(tc.tile_pool(name="psum", bufs=1, space="PSUM"))

        # ---- persistent SBUF ----
        w3_sb = singles.tile([P, KC, 3 * GD], bf16)
        wo_sb = singles.tile([P, 2, D], bf16)
        bq_sb = singles.tile([P, 2, 1], f32)
        bk_sb = singles.tile([P, 2, 1], f32)
        xq_sb = singles.tile([P, KC, S], bf16)
        xk_sb = singles.tile([P, KC, S], bf16)
        xv_sb = singles.tile([P, KC, S], bf16)
        qT_sb = [singles.tile([P, S], bf16, name=f"qT{t}") for t in range(2)]
        kT_sb = [singles.tile([P, S], bf16, name=f"kT{t}") for t in range(2)]
        # v_aug: per (j-in-tile, jt, head): [v_h | 1]
        v_sb = singles.tile([P, ST, HPG * 65], bf16)
        v4 = v_sb.rearrange("p s (h c) -> p s h c", c=65)
        nc.vector.memset(v4[:, :, :, 64:65], 1.0)
        # attT, ready for out-proj: row p = hp*64+e of pair c, col = i
        att2 = singles.tile([P, 2, S], bf16)

        # ---- input DMAs, ordered by consumer deadline (DMA_ENGINES
        # transfers serialize in emission order) ----
        w3_r = wqkv.rearrange("(c p) m -> p c m", p=P)
        xq_r = xqT.rearrange("(c p) s -> p c s", p=P)
        xk_r = xkT.rearrange("(c p) s -> p c s", p=P)
        xv_r = xvT.rearrange("(c p) s -> p c s", p=P)
        QS = S // 4

        def ldx(dst, src, q0, q1):
            nc.sync.dma_start(out=dst[:, :, q0 * QS:q1 * QS],
                              in_=src[:, :, q0 * QS:q1 * QS])

        ident_sb = singles.tile([P, P], bf16)
        nc.sync.dma_start(out=w3_sb[:, :, 0:2 * GD], in_=w3_r[:, :, 0:2 * GD])
        nc.sync.dma_start(out=bq_sb, in_=bq.rearrange("(t p) o -> p t o", p=P))
        nc.sync.dma_start(out=bk_sb, in_=bk.rearrange("(t p) o -> p t o", p=P))
        nc.sync.dma_start(out=ident_sb, in_=ident[:, :])
        ldx(xq_sb, xq_r, 0, 1)
        ldx(xq_sb, xq_r, 1, 2)
        ldx(xk_sb, xk_r, 0, 1)
        nc.sync.dma_start(out=w3_sb[:, :, 2 * GD:3 * GD],
                          in_=w3_r[:, :, 2 * GD:3 * GD])
        ldx(xv_sb, xv_r, 0, 1)
        ldx(xk_sb, xk_r, 1, 2)
        ldx(xv_sb, xv_r, 1, 2)
        ldx(xk_sb, xk_r, 2, 3)
        ldx(xv_sb, xv_r, 2, 3)
        ldx(xk_sb, xk_r, 3, 4)
        ldx(xv_sb, xv_r, 3, 4)
        ldx(xq_sb, xq_r, 2, 4)
        nc.sync.dma_start(out=wo_sb, in_=wo.rearrange("(c p) n -> p c n", p=P))

        WQ, WK, WV = 0, GD, 2 * GD  # column offsets into w3_sb

        # ---- generator extras: aux-slot work in <=430ns chunks ----
        def g_qk_chain(x_sb, w_off, b_sb, dstT, t, sblk):
            """One [128, 512] column block of a q/k projection."""
            a = psum.tile([P, 512], f32, tag="aux", bufs=1, name="pqa")
            sl = slice(sblk * 512, (sblk + 1) * 512)
            for k2 in range(0, KC, 2):
                for k in (k2, k2 + 1):
                    nc.tensor.matmul(
                        out=a,
                        lhsT=w3_sb[:, k, w_off + t * P:w_off + (t + 1) * P],
                        rhs=x_sb[:, k, sl],
                        start=(k == 0), stop=(k == KC - 1))
                if k2 == KC - 2:
                    # emit the bias-add INSIDE the final chunk: consumers
                    # of dstT emitted in later units must see this write
                    # (Tile dependencies are emission-ordered)
                    nc.vector.tensor_scalar_add(out=dstT[t][:, sl], in0=a,
                                                scalar1=b_sb[:, t, :])
                yield

        def g_v_rot(pair, r):
            """V projection for head pair `pair`, s-tiles 4r..4r+3, with a
            DVE copy out after every second s-tile."""
            a = psum.tile([P, 512], f32, tag="aux", bufs=1, name="pva")
            for q in range(4):
                st = 4 * r + q
                for k4 in range(0, KC, 4):
                    for k in range(k4, k4 + 4):
                        nc.tensor.matmul(
                            out=a[:, q * P:(q + 1) * P],
                            lhsT=xv_sb[:, k, st * P:(st + 1) * P],
                            rhs=w3_sb[:, k, WV + pair * P:WV + (pair + 1) * P],
                            start=(k == 0), stop=(k == KC - 1))
                    if k4 == KC - 4 and q % 2 == 1:
                        # copy emitted INSIDE the final chunk of each 2-st
                        # group, before the yield (emission-order deps)
                        nc.vector.tensor_copy(
                            out=v4[:, st - 1:st + 1,
                                   2 * pair:2 * pair + 2, 0:64],
                            in_=a[:, (q - 1) * P:(q + 1) * P].rearrange(
                                "p (q2 h e) -> p q2 h e", q2=2, h=2))
                    yield

        def g_po(st):
            """Out-projection for s-tile st (overlaps the last block). The
            out-DMA is emitted one chunk after the copy so the SP sequencer
            never parks on a long data wait (that would delay the
            transposes queued behind it)."""
            osb = outs_pool.tile([P, D], bf16, tag="osb", name="osb")
            for nb in range(2):
                a = psum.tile([P, 512], f32, tag="aux", bufs=1, name="poa")
                for c in range(2):
                    nc.tensor.matmul(
                        out=a,
                        lhsT=att2[:, c, st * P:(st + 1) * P],
                        rhs=wo_sb[:, c, nb * 512:(nb + 1) * 512],
                        start=(c == 0), stop=(c == 1))
                    yield
                nc.vector.tensor_copy(out=osb[:, nb * 512:(nb + 1) * 512],
                                      in_=a)
                yield
            yield
            nc.sync.dma_start(out=out[st * P:(st + 1) * P, :], in_=osb)

        # ---- PE warm-up: the TensorE p-state ramps over ~3us of continuous
        # work; the input DMAs take ~6us, so burn the wait on dummy matmuls
        # (into the aux psum slot, source is a memset tile) so the first
        # projection chains dispatch at full clock ----
        wsrc = singles.tile([P, 512], bf16)
        nc.vector.memset(wsrc, 0.03)
        wps = psum.tile([P, 512], f32, tag="aux", bufs=1, name="warm")
        for i in range(52):
            nc.tensor.matmul(out=wps[:, 0:P], lhsT=wsrc[:, 0:P],
                             rhs=wsrc[:, 0:P], start=True, stop=True)
        for i in range(4):
            nc.tensor.matmul(out=wps, lhsT=wsrc[:, 0:P], rhs=wsrc,
                             start=True, stop=True)

        # ---- prologue: q0 both half-0 chains + k0 chain 0 through the sc
        # slots (k0 chain 1 rides the aux slot early in block 0) ----
        pq0 = psum.tile([P, 1024], f32, tag="sc", bufs=2, name="pq0")
        pk0 = psum.tile([P, 1024], f32, tag="sc", bufs=2, name="pk0")

        def proto_chain(x_sb, w_off, b_sb, dstT, pq, sblk):
            sl = slice(sblk * 512, (sblk + 1) * 512)
            for k in range(KC):
                nc.tensor.matmul(
                    out=pq[:, sl],
                    lhsT=w3_sb[:, k, w_off:w_off + P],
                    rhs=x_sb[:, k, sl],
                    start=(k == 0), stop=(k == KC - 1))
            nc.vector.tensor_scalar_add(out=dstT[0][:, sl], in0=pq[:, sl],
                                        scalar1=b_sb[:, 0, :])

        proto_chain(xq_sb, WQ, bq_sb, qT_sb, pq0, 0)
        proto_chain(xq_sb, WQ, bq_sb, qT_sb, pq0, 1)
        proto_chain(xk_sb, WK, bk_sb, kT_sb, pk0, 0)

        # pv accumulators: allocated ONCE; 16 units of [128, 65] packed 7/7/2
        # into 3 banks (never straddling a 2KB bank). Unit u = it*2 + hp so
        # ascending-u runs are contiguous in the normalize output layout.
        # Cross-block reuse synchronizes through subtile WAR deps.
        pv = psum.tile([P, 3, 512], f32, tag="pv", bufs=1, name="pv")

        def pv_ap(u):
            b, s2 = divmod(u, 7)
            return pv[:, b, 65 * s2:65 * s2 + 65]

        def g_transp(nrm, tpair, tih):
            """PE-transpose a finished block's normalized attn into att2
            (chunked; runs as an early extra of the following block)."""
            tpt = psum.tile([P, 8, P], bf16, tag="aux", bufs=1, name="tpt")
            for t in range(8):
                nc.tensor.transpose(
                    out=tpt[:, t, :],
                    in_=nrm[:, 2 * t:2 * t + 2, :].rearrange(
                        "p a e -> p (a e)"),
                    identity=ident_sb)
                nc.vector.tensor_copy(
                    out=att2[:, tpair, tih * 1024 + t * P:
                             tih * 1024 + (t + 1) * P],
                    in_=tpt[:, t, :])
                yield

        # ---- attention block: software-pipelined (jt, hp) units ----
        def attention(pair, ih, extras, prev_norm, pre_extras=()):
            if prev_norm is not None:
                nrm_prev, ppair, pih = prev_norm()
                extras.appendleft((1, g_transp(nrm_prev, ppair, pih)))

            units = [(jt, hp) for jt in range(ST) for hp in range(2)]
            sc_tiles = {}

            def emit_scores(n):
                jt, hp = units[n]
                sct = psum.tile([P, 1024], f32, tag="sc", bufs=2, name="sct")
                for iq in range(2):
                    nc.tensor.matmul(
                        out=sct[:, iq * 512:(iq + 1) * 512],
                        lhsT=kT_sb[pair][hp * 64:(hp + 1) * 64,
                                         jt * P:(jt + 1) * P],
                        rhs=qT_sb[pair][hp * 64:(hp + 1) * 64,
                                        ih * 1024 + iq * 512:
                                        ih * 1024 + (iq + 1) * 512],
                        start=True, stop=True)
                sc_tiles[n] = sct

            emit_scores(0)
            emit_scores(1)
            for n, (jt, hp) in enumerate(units):
                pr = probs_pool.tile([P, 1024], bf16, tag="probs", name="pr")
                nc.scalar.activation(out=pr, in_=sc_tiles.pop(n),
                                     func=Act.Exp, scale=0.125)
                if n + 2 < len(units):
                    emit_scores(n + 2)
                if n == 0:
                    for fn in pre_extras:
                        fn()
                h = 2 * pair + hp
                for it in range(8):
                    u = it * 2 + hp
                    # start=True clears has_written for the WHOLE psum bank,
                    # so only the first matmul touching each bank may carry
                    # it (units 0/8/14 lead banks 0/1/2 in emission order);
                    # every other unit's first write lands on cleared
                    # has_written bits and overwrites, later ones accumulate
                    nc.tensor.matmul(
                        out=pv_ap(u),
                        lhsT=pr[:, it * P:(it + 1) * P],
                        rhs=v_sb[:, jt, h * 65:h * 65 + 65],
                        start=(jt == 0 and u in (0, 8, 14)),
                        stop=(jt == ST - 1))
                budget = 2
                while extras and extras[0][0] <= n and budget > 0:
                    try:
                        next(extras[0][1])
                        budget -= 1
                    except StopIteration:
                        extras.popleft()

            # drain leftovers (producers must be emitted before the next
            # block's consumers)
            while extras:
                try:
                    next(extras[0][1])
                except StopIteration:
                    extras.popleft()

            def norm():
                # computes nrm = pv * (1/den); the transpose of nrm into
                # att2 is done separately with PE transposes (dma_transpose
                # writes are invisible to Tile's dependency tracking, which
                # races readers against the xbar write)
                r = small.tile([P, 16, 1], f32, tag="r", name="r")
                pvb0 = pv[:, 0, 0:455].rearrange("p (s c) -> p s c", c=65)
                pvb1 = pv[:, 1, 0:455].rearrange("p (s c) -> p s c", c=65)
                pvb2 = pv[:, 2, 0:130].rearrange("p (s c) -> p s c", c=65)
                nc.vector.reciprocal(out=r[:, 0:7, 0], in_=pvb0[:, :, 64])
                nc.vector.reciprocal(out=r[:, 7:14, 0], in_=pvb1[:, :, 64])
                nc.vector.reciprocal(out=r[:, 14:16, 0], in_=pvb2[:, :, 64])
                nrm = norm_pool.tile([P, 16, 64], bf16, tag="norm",
                                     name="nrm")

                def bmul(dst, in0, rsl):
                    a1, a2 = bass.broadcast_tensor_aps(in0, rsl)
                    nc.vector.tensor_tensor(out=dst, in0=a1, in1=a2,
                                            op=Alu.mult)

                bmul(nrm[:, 0:7, :], pvb0[:, :, 0:64], r[:, 0:7, :])
                bmul(nrm[:, 7:8, :], pvb1[:, 0:1, 0:64], r[:, 7:8, :])
                bmul(nrm[:, 8:14, :], pvb1[:, 1:7, 0:64], r[:, 8:14, :])
                bmul(nrm[:, 14:16, :], pvb2[:, :, 0:64], r[:, 14:16, :])
                return nrm, pair, ih

            return norm

        # extras: (earliest_unit, generator), ordered by DMA arrival and
        # consumer deadline; at most 2 chunks advanced per unit.
        ex0 = deque([
            (4, g_qk_chain(xk_sb, WK, bk_sb, kT_sb, 0, 1)),
            (6, g_v_rot(0, 1)),
            (10, g_qk_chain(xk_sb, WK, bk_sb, kT_sb, 0, 2)),
            (12, g_v_rot(0, 2)),
            (16, g_qk_chain(xk_sb, WK, bk_sb, kT_sb, 0, 3)),
            (18, g_v_rot(0, 3)),
            (24, g_qk_chain(xq_sb, WQ, bq_sb, qT_sb, 0, 2)),
            (27, g_qk_chain(xq_sb, WQ, bq_sb, qT_sb, 0, 3)),
        ])
        ex1 = deque([
            (0, g_qk_chain(xq_sb, WQ, bq_sb, qT_sb, 1, 0)),
            (2, g_qk_chain(xk_sb, WK, bk_sb, kT_sb, 1, 0)),
            (4, g_qk_chain(xq_sb, WQ, bq_sb, qT_sb, 1, 1)),
            (6, g_v_rot(1, 0)),
            (9, g_qk_chain(xk_sb, WK, bk_sb, kT_sb, 1, 1)),
            (12, g_v_rot(1, 1)),
            (15, g_qk_chain(xk_sb, WK, bk_sb, kT_sb, 1, 2)),
            (18, g_qk_chain(xk_sb, WK, bk_sb, kT_sb, 1, 3)),
        ])
        ex2 = deque([
            (0, g_v_rot(1, 2)),
            (4, g_v_rot(1, 3)),
            (8, g_qk_chain(xq_sb, WQ, bq_sb, qT_sb, 1, 2)),
            (12, g_qk_chain(xq_sb, WQ, bq_sb, qT_sb, 1, 3)),
        ])
        ex3 = deque([(2 + 3 * i, g_po(st)) for i, st in enumerate(range(6))])

        def v00():
            for _ in g_v_rot(0, 0):
                pass

        po_rest = []
        if SERIAL:
            # bisection aid: drain every generator up front, no interleaving
            v00()
            for _, g in [*ex0, *ex1, *ex2]:
                for _ in g:
                    pass
            po_rest = [g for _, g in ex3]
            ex0, ex1, ex2, ex3 = deque(), deque(), deque(), deque()
        pending = attention(0, 0, ex0, None,
                            pre_extras=([] if SERIAL else [v00]))
        pending = attention(0, 1, ex1, pending)
        if DEBUG:
            dbg_early = nc.dram_tensor("dbg_early", [P, 1024], bf16,
                                       kind="ExternalOutput")
            nc.sync.dma_start(out=dbg_early[:, :], in_=att2[:, 0, 0:1024])
        pending = attention(1, 0, ex2, pending)
        pending = attention(1, 1, ex3, pending)
        for g in po_rest:
            for _ in g:
                pass
        nrm_last, _, _ = pending()
        if DEBUG:
            dbg_pv = nc.dram_tensor("dbg_pv", [P, 3 * 512], f32,
                                    kind="ExternalOutput")
            pvstage = singles.tile([P, 3, 512], f32)
            nc.vector.tensor_copy(out=pvstage, in_=pv)
            nc.sync.dma_start(out=dbg_pv[:, :],
                              in_=pvstage.rearrange("p a b -> p (a b)"))

        # ---- tail: out-projection for s-tiles 6..15 through a 7-slot PSUM
        # pipeline over the now-idle sc/pv banks, with the final block's
        # att2 assembly done by PE transposes into the aux bank (no
        # DMA-queue slot guards; PE never idles, so it keeps its full
        # p-state). s-tiles 6,7 need only the ih0 half of att2, so their
        # matmuls run DURING the final normalize; s-tile 8+t consumes
        # exactly transpose t, so everything pipelines. Copies alternate
        # ScalarE/DVE; out-DMAs are emitted late so the SP sequencer never
        # parks on a data wait.
        scA = psum.tile([P, 1024], f32, tag="sc", bufs=2, name="tscA")
        scB = psum.tile([P, 1024], f32, tag="sc", bufs=2, name="tscB")
        auxT = psum.tile([P, 8, P], bf16, tag="aux", bufs=1, name="taux")
        slots = [scA[:, 0:512], scA[:, 512:1024],
                 scB[:, 0:512], scB[:, 512:1024],
                 pv[:, 0, 0:512], pv[:, 1, 0:512], pv[:, 2, 0:512]]
        osbs = {}
        dma_queue = []

        def po_group(i, st, nb):
            slot = slots[i % 7]
            for c in range(2):
                nc.tensor.matmul(
                    out=slot,
                    lhsT=att2[:, c, st * P:(st + 1) * P],
                    rhs=wo_sb[:, c, nb * 512:(nb + 1) * 512],
                    start=(c == 0), stop=(c == 1))
            if nb == 0:
                osbs[st] = outs_pool.tile([P, D], bf16, tag="osb",
                                          name="osbt")
            o = osbs[st]
            if st % 2 == 0:
                nc.scalar.copy(out=o[:, nb * 512:(nb + 1) * 512], in_=slot)
            else:
                nc.vector.tensor_copy(out=o[:, nb * 512:(nb + 1) * 512],
                                      in_=slot)
            if nb == 1:
                dma_queue.append(st)
                if len(dma_queue) > 1:
                    st_d = dma_queue.pop(0)
                    nc.sync.dma_start(out=out[st_d * P:(st_d + 1) * P, :],
                                      in_=osbs.pop(st_d))

        gi = 0
        for st in (6, 7):
            for nb in range(2):
                po_group(gi, st, nb)
                gi += 1
        # PE transposes of the final block's normalized attn: t covers
        # i-tile 8+t; copies chase through the 4 aux sub-slots
        for t in range(8):
            tp = auxT[:, t, :]
            nc.tensor.transpose(
                out=tp,
                in_=nrm_last[:, 2 * t:2 * t + 2, :].rearrange(
                    "p a e -> p (a e)"),
                identity=ident_sb)
            dst = att2[:, 1, 1024 + t * P:1024 + (t + 1) * P]
            if t % 2 == 0:
                nc.scalar.copy(out=dst, in_=tp)
            else:
                nc.vector.tensor_copy(out=dst, in_=tp)
        for st in range(8, 16):
            for nb in range(2):
                po_group(gi, st, nb)
                gi += 1
        for st_d in dma_queue:
            nc.sync.dma_start(out=out[st_d * P:(st_d + 1) * P, :],
                              in_=osbs.pop(st_d))

        if DEBUG:
            dbg_att2 = nc.dram_tensor("dbg_att2", [P, 2 * S], bf16,
                                      kind="ExternalOutput")
            dbg_qk = nc.dram_tensor("dbg_qk", [P, 4 * S], bf16,
                                    kind="ExternalOutput")
            dbg_v = nc.dram_tensor("dbg_v", [P, ST * HPG * 65], bf16,
                                   kind="ExternalOutput")
            nc.sync.dma_start(out=dbg_att2[:, :],
                              in_=att2.rearrange("p a b -> p (a b)"))
            nc.sync.dma_start(out=dbg_qk[:, 0:S], in_=qT_sb[0])
            nc.sync.dma_start(out=dbg_qk[:, S:2 * S], in_=qT_sb[1])
            nc.sync.dma_start(out=dbg_qk[:, 2 * S:3 * S], in_=kT_sb[0])
            nc.sync.dma_start(out=dbg_qk[:, 3 * S:4 * S], in_=kT_sb[1])
            nc.sync.dma_start(out=dbg_v[:, :],
                              in_=v_sb.rearrange("p a b -> p (a b)"))

    nc.finalize()
    return nc


def kernel(Q, K, V, Wq, bq, Wk, bk, Wv, bv, Wo, bo):
    from concourse.bass_utils import run_bass_kernel_spmd

    f32 = np.float32
    Q = np.asarray(Q, f32)
    K = np.asarray(K, f32)
    V = np.asarray(V, f32)
    Wq = np.asarray(Wq, f32)
    Wk = np.asarray(Wk, f32)
    Wv = np.asarray(Wv, f32)
    Wo = np.asarray(Wo, f32)
    bq = np.asarray(bq, f32)
    bk = np.asarray(bk, f32)
    bv = np.asarray(bv, f32)
    bo = np.asarray(bo, f32)

    xT = {}
    for b in range(B):
        xT[('q', b)] = np.ascontiguousarray(Q[b].T).astype(BF16)
        xT[('k', b)] = np.ascontiguousarray(K[b].T).astype(BF16)
        xT[('v', b)] = np.ascontiguousarray(V[b].T).astype(BF16)

    in_maps = []
    for c in range(NCORES):
        b, g = c // GROUPS, c % GROUPS
        sl = slice(g * GD, (g + 1) * GD)
        wqkv = np.concatenate([Wq[:, sl], Wk[:, sl], Wv[:, sl]], axis=1)
        in_maps.append({
            "xqT": xT[('q', b)],
            "xkT": xT[('k', b)],
            "xvT": xT[('v', b)],
            "wqkv": np.ascontiguousarray(wqkv).astype(BF16),
            "wo": np.ascontiguousarray(Wo[sl, :]).astype(BF16),
            "bq": np.ascontiguousarray(bq[sl].reshape(GD, 1)),
            "bk": np.ascontiguousarray(bk[sl].reshape(GD, 1)),
            "ident": np.eye(P, dtype=BF16),
        })

    if "nc" not in _cached:
        _cached["nc"] = _build_bass()
    nc = _cached["nc"]

    try:
        res = run_bass_kernel_spmd(nc, in_maps, core_ids=list(range(NCORES)))
    except ModuleNotFoundError:
        # BASS_TRACE set but the axon ntff hook isn't shipped in this
        # container - retry untraced
        os.environ["BASS_NEVER_TRACE"] = "1"
        res = run_bass_kernel_spmd(nc, in_maps, core_ids=list(range(NCORES)))
    if res.exec_time_ns is not None:
        print(f"HW exec time: {res.exec_time_ns} ns")

    bo_eff = (bv @ Wo + bo).astype(f32)
    out = np.zeros((B, S, D), f32)
    for c in range(NCORES):
        b = c // GROUPS
        out[b] += np.asarray(res.results[c]["out"]).astype(f32)
    out += bo_eff
    return out


# revision 53
# speedup vs baseline: 1.3410x; 1.0068x over previous
"""Multi-head attention (B=2, S=2048, D=1024, H=16, dk=64) on 8 TRN2 cores.

Sharding: core c -> (batch b = c//4, head-group g = c%4 of 4 heads).
Each core computes q/k/v projections for its 4 heads, full attention for
those heads, and a partial output projection (rows g*256:(g+1)*256 of Wo).
Host pre-transposes/casts inputs to bf16 and sums the partial outputs.

Structure (per core, matmul operands bf16, accumulation f32):
  xqT/xkT/xvT [1024, 2048]  (d on partitions -> contraction-ready)
  qT, kT      [128, 2048]   per head pair (head-dim on partitions)
  v4          [128 j, 16 jt, 4h x 65]  (per head: [v_h | ones]; ones col
              makes PV's 65th output column the softmax denominator)
  scoresT     sc[j, i-block] in PSUM (2 banks); exp on ScalarE -> probs
              [j, i] bf16 (no max-subtract: scores/8 ~ N(0,1))
  PV          pv[i, e] += probs[:, it-slice].T @ v_aug  (N=65 matmuls,
              accumulated over 16 j-tiles; col 64 = denominator)
  normalize   DVE: r = 1/den; attn = pv * r (broadcast tensor_tensor over
              runs of units)
  transpose   DMA xbar: attn [i, e] -> att2 [e2=(hp,e), pair, i]
  out-projT   out[s, n] = sum_c att2[:, c, s-slice].T @ wo[c]  (K=256)

The jt loop is software-pipelined so ScalarE exp calls run back-to-back
(the bottleneck engine). All other TensorE work (later head-pair q/k
projections, the V projection, the output projection) is chopped into
<=430ns chunks by generators and drip-fed into TensorE's slack through a
single rotating PSUM "aux" bank. Each generator carries an
earliest-unit gate matched to the input-DMA arrival order, and producers
are always EMITTED before their consumers (Tile's dependency tracking is
emission-ordered). Host: out[b] = sum_g outT_partial + (bv @ Wo + bo).
"""

import os
from collections import deque

import numpy as np
import ml_dtypes

BF16 = ml_dtypes.bfloat16

B, S, D = 2, 2048, 1024
H, DK = 16, 64
P = 128
GROUPS = 4          # head groups (one per core within a batch)
HPG = 4             # heads per group
GD = HPG * DK       # 256, group width
KC = D // P         # 8 contraction chunks
ST = S // P         # 16 s-tiles / j-tiles
NCORES = 8
DEBUG = False
SERIAL = False

_cached = {}


def _build_bass():
    import concourse.bass as bass
    import concourse.tile as tile
    from concourse.bacc import Bacc
    from concourse import mybir
    from contextlib import ExitStack

    f32 = mybir.dt.float32
    bf16 = mybir.dt.bfloat16
    Act = mybir.ActivationFunctionType
    Alu = mybir.AluOpType

    nc = Bacc()

    xqT = nc.dram_tensor("xqT", [D, S], bf16, kind="ExternalInput")
    xkT = nc.dram_tensor("xkT", [D, S], bf16, kind="ExternalInput")
    xvT = nc.dram_tensor("xvT", [D, S], bf16, kind="ExternalInput")
    # wq | wk | wv concatenated column-wise
    wqkv = nc.dram_tensor("wqkv", [D, 3 * GD], bf16, kind="ExternalInput")
    wo = nc.dram_tensor("wo", [GD, D], bf16, kind="ExternalInput")
    bq = nc.dram_tensor("bq", [GD, 1], f32, kind="ExternalInput")
    bk = nc.dram_tensor("bk", [GD, 1], f32, kind="ExternalInput")
    ident = nc.dram_tensor("ident", [P, P], bf16, kind="ExternalInput")
    out = nc.dram_tensor("out", [S, D], bf16, kind="ExternalOutput")

    with tile.TileContext(nc) as tc, ExitStack() as ctx:
        singles = ctx.enter_context(tc.tile_pool(name="singles", bufs=1))
        probs_pool = ctx.enter_context(tc.tile_pool(name="probs", bufs=4))
        norm_pool = ctx.enter_context(tc.tile_pool(name="normp", bufs=2))
        small = ctx.enter_context(tc.tile_pool(name="small", bufs=2))
        outs_pool = ctx.enter_context(tc.tile_pool(name="outs", bufs=4))
        psum = ctx.enter_context(tc.tile_pool(name="psum", bufs=1, space="PSUM"))

        # ---- persistent SBUF ----
        w3_sb = singles.tile([P, KC, 3 * GD], bf16)
        wo_sb = singles.tile([P, 2, D], bf16)
        bq_sb = singles.tile([P, 2, 1], f32)
        bk_sb = singles.tile([P, 2, 1], f32)
        xq_sb = singles.tile([P, KC, S], bf16)
        xk_sb = singles.tile([P, KC, S], bf16)
        xv_sb = singles.tile([P, KC, S], bf16)
        qT_sb = [singles.tile([P, S], bf16, name=f"qT{t}") for t in range(2)]
        kT_sb = [singles.tile([P, S], bf16, name=f"kT{t}") for t in range(2)]
        # v_aug: per (j-in-tile, jt, head): [v_h | 1]
        v_sb = singles.tile([P, ST, HPG * 65], bf16)
        v4 = v_sb.rearrange("p s (h c) -> p s h c", c=65)
        nc.vector.memset(v4[:, :, :, 64:65], 1.0)
        # attT, ready for out-proj: row p = hp*64+e of pair c, col = i
        att2 = singles.tile([P, 2, S], bf16)

        # ---- input DMAs, ordered by consumer deadline (DMA_ENGINES
        # transfers serialize in emission order) ----
        w3_r = wqkv.rearrange("(c p) m -> p c m", p=P)
        xq_r = xqT.rearrange("(c p) s -> p c s", p=P)
        xk_r = xkT.rearrange("(c p) s -> p c s", p=P)
        xv_r = xvT.rearrange("(c p) s -> p c s", p=P)
        QS = S // 4

        def ldx(dst, src, q0, q1):
            nc.sync.dma_start(out=dst[:, :, q0 * QS:q1 * QS],
                              in_=src[:, :, q0 * QS:q1 * QS])

        ident_sb = singles.tile([P, P], bf16)
        nc.sync.dma_start(out=w3_sb[:, :, 0:2 * GD], in_=w3_r[:, :, 0:2 * GD])
        nc.sync.dma_start(out=bq_sb, in_=bq.rearrange("(t p) o -> p t o", p=P))
        nc.sync.dma_start(out=bk_sb, in_=bk.rearrange("(t p) o -> p t o", p=P))
        nc.sync.dma_start(out=ident_sb, in_=ident[:, :])
        ldx(xq_sb, xq_r, 0, 1)
        ldx(xq_sb, xq_r, 1, 2)
        ldx(xk_sb, xk_r, 0, 1)
        nc.sync.dma_start(out=w3_sb[:, :, 2 * GD:3 * GD],
                          in_=w3_r[:, :, 2 * GD:3 * GD])
        ldx(xv_sb, xv_r, 0, 1)
        ldx(xk_sb, xk_r, 1, 2)
        ldx(xv_sb, xv_r, 1, 2)
        ldx(xk_sb, xk_r, 2, 3)
        ldx(xv_sb, xv_r, 2, 3)
        ldx(xk_sb, xk_r, 3, 4)
        ldx(xv_sb, xv_r, 3, 4)
        ldx(xq_sb, xq_r, 2, 4)
        nc.sync.dma_start(out=wo_sb, in_=wo.rearrange("(c p) n -> p c n", p=P))

        WQ, WK, WV = 0, GD, 2 * GD  # column offsets into w3_sb

        # ---- generator extras: aux-slot work in <=430ns chunks ----
        def g_qk_chain(x_sb, w_off, b_sb, dstT, t, sblk):
            """One [128, 512] column block of a q/k projection."""
            a = psum.tile([P, 512], f32, tag="aux", bufs=1, name="pqa")
            sl = slice(sblk * 512, (sblk + 1) * 512)
            for k2 in range(0, KC, 2):
                for k in (k2, k2 + 1):
                    nc.tensor.matmul(
                        out=a,
                        lhsT=w3_sb[:, k, w_off + t * P:w_off + (t + 1) * P],
                        rhs=x_sb[:, k, sl],
                        start=(k == 0), stop=(k == KC - 1))
                if k2 == KC - 2:
                    # emit the bias-add INSIDE the final chunk: consumers
                    # of dstT emitted in later units must see this write
                    # (Tile dependencies are emission-ordered)
                    nc.vector.tensor_scalar_add(out=dstT[t][:, sl], in0=a,
                                                scalar1=b_sb[:, t, :])
                yield

        def g_v_rot(pair, r):
            """V projection for head pair `pair`, s-tiles 4r..4r+3, with a
            DVE copy out after every second s-tile."""
            a = psum.tile([P, 512], f32, tag="aux", bufs=1, name="pva")
            for q in range(4):
                st = 4 * r + q
                for k4 in range(0, KC, 4):
                    for k in range(k4, k4 + 4):
                        nc.tensor.matmul(
                            out=a[:, q * P:(q + 1) * P],
                            lhsT=xv_sb[:, k, st * P:(st + 1) * P],
                            rhs=w3_sb[:, k, WV + pair * P:WV + (pair + 1) * P],
                            start=(k == 0), stop=(k == KC - 1))
                    if k4 == KC - 4 and q % 2 == 1:
                        # copy emitted INSIDE the final chunk of each 2-st
                        # group, before the yield (emission-order deps)
                        nc.vector.tensor_copy(
                            out=v4[:, st - 1:st + 1,
                                   2 * pair:2 * pair + 2, 0:64],
                            in_=a[:, (q - 1) * P:(q + 1) * P].rearrange(
                                "p (q2 h e) -> p q2 h e", q2=2, h=2))
                    yield

        def g_po(st):
            """Out-projection for s-tile st (overlaps the last block). The
            out-DMA is emitted one chunk after the copy so the SP sequencer
            never parks on a long data wait (that would delay the
            transposes queued behind it)."""
            osb = outs_pool.tile([P, D], bf16, tag="osb", name="osb")
            for nb in range(2):
                a = psum.tile([P, 512], f32, tag="aux", bufs=1, name="poa")
                for c in range(2):
                    nc.tensor.matmul(
                        out=a,
                        lhsT=att2[:, c, st * P:(st + 1) * P],
                        rhs=wo_sb[:, c, nb * 512:(nb + 1) * 512],
                        start=(c == 0), stop=(c == 1))
                    yield
                nc.vector.tensor_copy(out=osb[:, nb * 512:(nb + 1) * 512],
                                      in_=a)
                yield
            yield
            nc.sync.dma_start(out=out[st * P:(st + 1) * P, :], in_=osb)

        # ---- PE warm-up: the TensorE p-state ramps over ~3us of continuous
        # work; the input DMAs take ~6us, so burn the wait on dummy matmuls
        # (into the aux psum slot, source is a memset tile) so the first
        # projection chains dispatch at full clock ----
        wsrc = singles.tile([P, 512], bf16)
        nc.vector.memset(wsrc, 0.03)
        wps = psum.tile([P, 512], f32, tag="aux", bufs=1, name="warm")
        for i in range(52):
            nc.tensor.matmul(out=wps[:, 0:P], lhsT=wsrc[:, 0:P],
                             rhs=wsrc[:, 0:P], start=True, stop=True)
        for i in range(4):
            nc.tensor.matmul(out=wps, lhsT=wsrc[:, 0:P], rhs=wsrc,
                             start=True, stop=True)

        # ---- prologue: q0 both half-0 chains + k0 chain 0 through the sc
        # slots (k0 chain 1 rides the aux slot early in block 0) ----
        pq0 = psum.tile([P, 1024], f32, tag="sc", bufs=2, name="pq0")
        pk0 = psum.tile([P, 1024], f32, tag="sc", bufs=2, name="pk0")

        def proto_chain(x_sb, w_off, b_sb, dstT, pq, sblk):
            sl = slice(sblk * 512, (sblk + 1) * 512)
            for k in range(KC):
                nc.tensor.matmul(
                    out=pq[:, sl],
                    lhsT=w3_sb[:, k, w_off:w_off + P],
                    rhs=x_sb[:, k, sl],
                    start=(k == 0), stop=(k == KC - 1))
            nc.vector.tensor_scalar_add(out=dstT[0][:, sl], in0=pq[:, sl],
                                        scalar1=b_sb[:, 0, :])

        proto_chain(xq_sb, WQ, bq_sb, qT_sb, pq0, 0)
        proto_chain(xq_sb, WQ, bq_sb, qT_sb, pq0, 1)
        proto_chain(xk_sb, WK, bk_sb, kT_sb, pk0, 0)

        # pv accumulators: allocated ONCE; 16 units of [128, 65] packed 7/7/2
        # into 3 banks (never straddling a 2KB bank). Unit u = it*2 + hp so
        # ascending-u runs are contiguous in the normalize output layout.
        # Cross-block reuse synchronizes through subtile WAR deps.
        pv = psum.tile([P, 3, 512], f32, tag="pv", bufs=1, name="pv")

        def pv_ap(u):
            b, s2 = divmod(u, 7)
            return pv[:, b, 65 * s2:65 * s2 + 65]

        def g_transp(nrm, tpair, tih):
            """PE-transpose a finished block's normalized attn into att2
            (chunked; runs as an early extra of the following block)."""
            tpt = psum.tile([P, 8, P], bf16, tag="aux", bufs=1, name="tpt")
            for t in range(8):
                nc.tensor.transpose(
                    out=tpt[:, t, :],
                    in_=nrm[:, 2 * t:2 * t + 2, :].rearrange(
                        "p a e -> p (a e)"),
                    identity=ident_sb)
                nc.vector.tensor_copy(
                    out=att2[:, tpair, tih * 1024 + t * P:
                             tih * 1024 + (t + 1) * P],
                    in_=tpt[:, t, :])
                yield

        # ---- attention block: software-pipelined (jt, hp) units ----
        def attention(pair, ih, extras, prev_norm, pre_extras=()):
            if prev_norm is not None:
                nrm_prev, ppair, pih = prev_norm()
                extras.appendleft((1, g_transp(nrm_prev, ppair, pih)))

            units = [(jt, hp) for jt in range(ST) for hp in range(2)]
            sc_tiles = {}

            def emit_scores(n):
                jt, hp = units[n]
                sct = psum.tile([P, 1024], f32, tag="sc", bufs=2, name="sct")
                for iq in range(2):
                    nc.tensor.matmul(
                        out=sct[:, iq * 512:(iq + 1) * 512],
                        lhsT=kT_sb[pair][hp * 64:(hp + 1) * 64,
                                         jt * P:(jt + 1) * P],
                        rhs=qT_sb[pair][hp * 64:(hp + 1) * 64,
                                        ih * 1024 + iq * 512:
                                        ih * 1024 + (iq + 1) * 512],
                        start=True, stop=True)
                sc_tiles[n] = sct

            emit_scores(0)
            emit_scores(1)
            for n, (jt, hp) in enumerate(units):
                pr = probs_pool.tile([P, 1024], bf16, tag="probs", name="pr")
                nc.scalar.activation(out=pr, in_=sc_tiles.pop(n),
                                     func=Act.Exp, scale=0.125)
                if n + 2 < len(units):
                    emit_scores(n + 2)
                if n == 0:
                    for fn in pre_extras:
                        fn()
                h = 2 * pair + hp
                for it in range(8):
                    u = it * 2 + hp
                    # start=True clears has_written for the WHOLE psum bank,
                    # so only the first matmul touching each bank may carry
                    # it (units 0/8/14 lead banks 0/1/2 in emission order);
                    # every other unit's first write lands on cleared
                    # has_written bits and overwrites, later ones accumulate
                    nc.tensor.matmul(
                        out=pv_ap(u),
                        lhsT=pr[:, it * P:(it + 1) * P],
                        rhs=v_sb[:, jt, h * 65:h * 65 + 65],
                        start=(jt == 0 and u in (0, 8, 14)),
                        stop=(jt == ST - 1))
                budget = 2
                while extras and extras[0][0] <= n and budget > 0:
                    try:
                        next(extras[0][1])
                        budget -= 1
                    except StopIteration:
                        extras.popleft()

            # drain leftovers (producers must be emitted before the next
            # block's consumers)
            while extras:
                try:
                    next(extras[0][1])
                except StopIteration:
                    extras.popleft()

            def norm():
                # computes nrm = pv * (1/den); the transpose of nrm into
                # att2 is done separately with PE transposes (dma_transpose
                # writes are invisible to Tile's dependency tracking, which
                # races readers against the xbar write)
                r = small.tile([P, 16, 1], f32, tag="r", name="r")
                pvb0 = pv[:, 0, 0:455].rearrange("p (s c) -> p s c", c=65)
                pvb1 = pv[:, 1, 0:455].rearrange("p (s c) -> p s c", c=65)
                pvb2 = pv[:, 2, 0:130].rearrange("p (s c) -> p s c", c=65)
                nc.vector.reciprocal(out=r[:, 0:7, 0], in_=pvb0[:, :, 64])
                nc.vector.reciprocal(out=r[:, 7:14, 0], in_=pvb1[:, :, 64])
                nc.vector.reciprocal(out=r[:, 14:16, 0], in_=pvb2[:, :, 64])
                nrm = norm_pool.tile([P, 16, 64], bf16, tag="norm",
                                     name="nrm")

                def bmul(dst, in0, rsl):
                    a1, a2 = bass.broadcast_tensor_aps(in0, rsl)
                    nc.vector.tensor_tensor(out=dst, in0=a1, in1=a2,
                                            op=Alu.mult)

                bmul(nrm[:, 0:7, :], pvb0[:, :, 0:64], r[:, 0:7, :])
                bmul(nrm[:, 7:8, :], pvb1[:, 0:1, 0:64], r[:, 7:8, :])
                bmul(nrm[:, 8:14, :], pvb1[:, 1:7, 0:64], r[:, 8:14, :])
                bmul(nrm[:, 14:16, :], pvb2[:, :, 0:64], r[:, 14:16, :])
                return nrm, pair, ih

            return norm

        # extras: (earliest_unit, generator), ordered by DMA arrival and
        # consumer deadline; at most 2 chunks advanced per unit.
        ex0 = deque([
            (4, g_qk_chain(xk_sb, WK, bk_sb, kT_sb, 0, 1)),
            (6, g_v_rot(0, 1)),
            (10, g_qk_chain(xk_sb, WK, bk_sb, kT_sb, 0, 2)),
            (12, g_v_rot(0, 2)),
            (16, g_qk_chain(xk_sb, WK, bk_sb, kT_sb, 0, 3)),
            (18, g_v_rot(0, 3)),
            (24, g_qk_chain(xq_sb, WQ, bq_sb, qT_sb, 0, 2)),
            (27, g_qk_chain(xq_sb, WQ, bq_sb, qT_sb, 0, 3)),
        ])
        ex1 = deque([
            (0, g_qk_chain(xq_sb, WQ, bq_sb, qT_sb, 1, 0)),
            (2, g_qk_chain(xk_sb, WK, bk_sb, kT_sb, 1, 0)),
            (4, g_qk_chain(xq_sb, WQ, bq_sb, qT_sb, 1, 1)),
            (6, g_v_rot(1, 0)),
            (9, g_qk_chain(xk_sb, WK, bk_sb, kT_sb, 1, 1)),
            (12, g_v_rot(1, 1)),
            (15, g_qk_chain(xk_sb, WK, bk_sb, kT_sb, 1, 2)),
            (18, g_qk_chain(xk_sb, WK, bk_sb, kT_sb, 1, 3)),
        ])
        ex2 = deque([
            (0, g_v_rot(1, 2)),
            (4, g_v_rot(1, 3)),
            (8, g_qk_chain(xq_sb, WQ, bq_sb, qT_sb, 1, 2)),
            (12, g_qk_chain(xq_sb, WQ, bq_sb, qT_sb, 1, 3)),
        ])
        ex3 = deque([(2 + 3 * i, g_po(st)) for i, st in enumerate(range(6))])

        def v00():
            for _ in g_v_rot(0, 0):
                pass

        po_rest = []
        if SERIAL:
            # bisection aid: drain every generator up front, no interleaving
            v00()
            for _, g in [*ex0, *ex1, *ex2]:
                for _ in g:
                    pass
            po_rest = [g for _, g in ex3]
            ex0, ex1, ex2, ex3 = deque(), deque(), deque(), deque()
        pending = attention(0, 0, ex0, None,
                            pre_extras=([] if SERIAL else [v00]))
        pending = attention(0, 1, ex1, pending)
        if DEBUG:
            dbg_early = nc.dram_tensor("dbg_early", [P, 1024], bf16,
                                       kind="ExternalOutput")
            nc.sync.dma_start(out=dbg_early[:, :], in_=att2[:, 0, 0:1024])
        pending = attention(1, 0, ex2, pending)
        pending = attention(1, 1, ex3, pending)
        for g in po_rest:
            for _ in g:
                pass
        nrm_last, _, _ = pending()
        if DEBUG:
            dbg_pv = nc.dram_tensor("dbg_pv", [P, 3 * 512], f32,
                                    kind="ExternalOutput")
            pvstage = singles.tile([P, 3, 512], f32)
            nc.vector.tensor_copy(out=pvstage, in_=pv)
            nc.sync.dma_start(out=dbg_pv[:, :],
                              in_=pvstage.rearrange("p a b -> p (a b)"))

        # ---- tail: out-projection for s-tiles 6..15 through a 7-slot PSUM
        # pipeline over the now-idle sc/pv banks, with the final block's
        # att2 assembly done by PE transposes into the aux bank (no
        # DMA-queue slot guards; PE never idles, so it keeps its full
        # p-state). s-tiles 6,7 need only the ih0 half of att2, so their
        # matmuls run DURING the final normalize; s-tile 8+t consumes
        # exactly transpose t, so everything pipelines. Copies alternate
        # ScalarE/DVE; out-DMAs are emitted late so the SP sequencer never
        # parks on a data wait.
        scA = psum.tile([P, 1024], f32, tag="sc", bufs=2, name="tscA")
        scB = psum.tile([P, 1024], f32, tag="sc", bufs=2, name="tscB")
        auxT = psum.tile([P, 8, P], bf16, tag="aux", bufs=1, name="taux")
        slots = [scA[:, 0:512], scA[:, 512:1024],
                 scB[:, 0:512], scB[:, 512:1024],
                 pv[:, 0, 0:512], pv[:, 1, 0:512], pv[:, 2, 0:512]]
        osbs = {}
        dma_queue = []

        def po_group(i, st, nb):
            slot = slots[i % 7]
            for c in range(2):
                nc.tensor.matmul(
                    out=slot,
                    lhsT=att2[:, c, st * P:(st + 1) * P],
                    rhs=wo_sb[:, c, nb * 512:(nb + 1) * 512],
                    start=(c == 0), stop=(c == 1))
            if nb == 0:
                osbs[st] = outs_pool.tile([P, D], bf16, tag="osb",
                                          name="osbt")
            o = osbs[st]
            # s-tiles 6/7 copy on ScalarE (it idles right after the last
            # exp, while DVE is busy with the final normalize)
            if st < 8 or st % 2 == 0:
                nc.scalar.copy(out=o[:, nb * 512:(nb + 1) * 512], in_=slot)
            else:
                nc.vector.tensor_copy(out=o[:, nb * 512:(nb + 1) * 512],
                                      in_=slot)
            if nb == 1:
                dma_queue.append(st)
                if len(dma_queue) > 1:
                    st_d = dma_queue.pop(0)
                    nc.sync.dma_start(out=out[st_d * P:(st_d + 1) * P, :],
                                      in_=osbs.pop(st_d))

        gi = 0
        for st in (6, 7):
            for nb in range(2):
                po_group(gi, st, nb)
                gi += 1
        # PE transposes of the final block's normalized attn: t covers
        # i-tile 8+t; copies chase through the 4 aux sub-slots
        for t in range(8):
            tp = auxT[:, t, :]
            nc.tensor.transpose(
                out=tp,
                in_=nrm_last[:, 2 * t:2 * t + 2, :].rearrange(
                    "p a e -> p (a e)"),
                identity=ident_sb)
            dst = att2[:, 1, 1024 + t * P:1024 + (t + 1) * P]
            if t % 2 == 0:
                nc.scalar.copy(out=dst, in_=tp)
            else:
                nc.vector.tensor_copy(out=dst, in_=tp)
        for st in range(8, 16):
            for nb in range(2):
                po_group(gi, st, nb)
                gi += 1
        for st_d in dma_queue:
            nc.sync.dma_start(out=out[st_d * P:(st_d + 1) * P, :],
                              in_=osbs.pop(st_d))

        if DEBUG:
            dbg_att2 = nc.dram_tensor("dbg_att2", [P, 2 * S], bf16,
                                      kind="ExternalOutput")
            dbg_qk = nc.dram_tensor("dbg_qk", [P, 4 * S], bf16,
                                    kind="ExternalOutput")
            dbg_v = nc.dram_tensor("dbg_v", [P, ST * HPG * 65], bf16,
                                   kind="ExternalOutput")
            nc.sync.dma_start(out=dbg_att2[:, :],
                              in_=att2.rearrange("p a b -> p (a b)"))
            nc.sync.dma_start(out=dbg_qk[:, 0:S], in_=qT_sb[0])
            nc.sync.dma_start(out=dbg_qk[:, S:2 * S], in_=qT_sb[1])
            nc.sync.dma_start(out=dbg_qk[:, 2 * S:3 * S], in_=kT_sb[0])
            nc.sync.dma_start(out=dbg_qk[:, 3 * S:4 * S], in_=kT_sb[1])
            nc.sync.dma_start(out=dbg_v[:, :],
                              in_=v_sb.rearrange("p a b -> p (a b)"))

    nc.finalize()
    return nc


def kernel(Q, K, V, Wq, bq, Wk, bk, Wv, bv, Wo, bo):
    from concourse.bass_utils import run_bass_kernel_spmd

    f32 = np.float32
    Q = np.asarray(Q, f32)
    K = np.asarray(K, f32)
    V = np.asarray(V, f32)
    Wq = np.asarray(Wq, f32)
    Wk = np.asarray(Wk, f32)
    Wv = np.asarray(Wv, f32)
    Wo = np.asarray(Wo, f32)
    bq = np.asarray(bq, f32)
    bk = np.asarray(bk, f32)
    bv = np.asarray(bv, f32)
    bo = np.asarray(bo, f32)

    xT = {}
    for b in range(B):
        xT[('q', b)] = np.ascontiguousarray(Q[b].T).astype(BF16)
        xT[('k', b)] = np.ascontiguousarray(K[b].T).astype(BF16)
        xT[('v', b)] = np.ascontiguousarray(V[b].T).astype(BF16)

    in_maps = []
    for c in range(NCORES):
        b, g = c // GROUPS, c % GROUPS
        sl = slice(g * GD, (g + 1) * GD)
        wqkv = np.concatenate([Wq[:, sl], Wk[:, sl], Wv[:, sl]], axis=1)
        in_maps.append({
            "xqT": xT[('q', b)],
            "xkT": xT[('k', b)],
            "xvT": xT[('v', b)],
            "wqkv": np.ascontiguousarray(wqkv).astype(BF16),
            "wo": np.ascontiguousarray(Wo[sl, :]).astype(BF16),
            "bq": np.ascontiguousarray(bq[sl].reshape(GD, 1)),
            "bk": np.ascontiguousarray(bk[sl].reshape(GD, 1)),
            "ident": np.eye(P, dtype=BF16),
        })

    if "nc" not in _cached:
        _cached["nc"] = _build_bass()
    nc = _cached["nc"]

    try:
        res = run_bass_kernel_spmd(nc, in_maps, core_ids=list(range(NCORES)))
    except ModuleNotFoundError:
        # BASS_TRACE set but the axon ntff hook isn't shipped in this
        # container - retry untraced
        os.environ["BASS_NEVER_TRACE"] = "1"
        res = run_bass_kernel_spmd(nc, in_maps, core_ids=list(range(NCORES)))
    if res.exec_time_ns is not None:
        print(f"HW exec time: {res.exec_time_ns} ns")

    bo_eff = (bv @ Wo + bo).astype(f32)
    out = np.zeros((B, S, D), f32)
    for c in range(NCORES):
        b = c // GROUPS
        out[b] += np.asarray(res.results[c]["out"]).astype(f32)
    out += bo_eff
    return out


# revision 54
# speedup vs baseline: 1.3471x; 1.0046x over previous
"""Multi-head attention (B=2, S=2048, D=1024, H=16, dk=64) on 8 TRN2 cores.

Sharding: core c -> (batch b = c//4, head-group g = c%4 of 4 heads).
Each core computes q/k/v projections for its 4 heads, full attention for
those heads, and a partial output projection (rows g*256:(g+1)*256 of Wo).
Host pre-transposes/casts inputs to bf16 and sums the partial outputs.

Structure (per core, matmul operands bf16, accumulation f32):
  xqT/xkT/xvT [1024, 2048]  (d on partitions -> contraction-ready)
  qT, kT      [128, 2048]   per head pair (head-dim on partitions)
  v4          [128 j, 16 jt, 4h x 65]  (per head: [v_h | ones]; ones col
              makes PV's 65th output column the softmax denominator)
  scoresT     sc[j, i-block] in PSUM (2 banks); exp on ScalarE -> probs
              [j, i] bf16 (no max-subtract: scores/8 ~ N(0,1))
  PV          pv[i, e] += probs[:, it-slice].T @ v_aug  (N=65 matmuls,
              accumulated over 16 j-tiles; col 64 = denominator)
  normalize   DVE: r = 1/den; attn = pv * r (broadcast tensor_tensor over
              runs of units)
  transpose   DMA xbar: attn [i, e] -> att2 [e2=(hp,e), pair, i]
  out-projT   out[s, n] = sum_c att2[:, c, s-slice].T @ wo[c]  (K=256)

The jt loop is software-pipelined so ScalarE exp calls run back-to-back
(the bottleneck engine). All other TensorE work (later head-pair q/k
projections, the V projection, the output projection) is chopped into
<=430ns chunks by generators and drip-fed into TensorE's slack through a
single rotating PSUM "aux" bank. Each generator carries an
earliest-unit gate matched to the input-DMA arrival order, and producers
are always EMITTED before their consumers (Tile's dependency tracking is
emission-ordered). Host: out[b] = sum_g outT_partial + (bv @ Wo + bo).
"""

import os
from collections import deque

import numpy as np
import ml_dtypes

BF16 = ml_dtypes.bfloat16

B, S, D = 2, 2048, 1024
H, DK = 16, 64
P = 128
GROUPS = 4          # head groups (one per core within a batch)
HPG = 4             # heads per group
GD = HPG * DK       # 256, group width
KC = D // P         # 8 contraction chunks
ST = S // P         # 16 s-tiles / j-tiles
NCORES = 8
DEBUG = False
SERIAL = False

_cached = {}


def _build_bass():
    import concourse.bass as bass
    import concourse.tile as tile
    from concourse.bacc import Bacc
    from concourse import mybir
    from contextlib import ExitStack

    f32 = mybir.dt.float32
    bf16 = mybir.dt.bfloat16
    Act = mybir.ActivationFunctionType
    Alu = mybir.AluOpType

    nc = Bacc()

    xqT = nc.dram_tensor("xqT", [D, S], bf16, kind="ExternalInput")
    xkT = nc.dram_tensor("xkT", [D, S], bf16, kind="ExternalInput")
    xvT = nc.dram_tensor("xvT", [D, S], bf16, kind="ExternalInput")
    # wq | wk | wv concatenated column-wise
    wqkv = nc.dram_tensor("wqkv", [D, 3 * GD], bf16, kind="ExternalInput")
    wo = nc.dram_tensor("wo", [GD, D], bf16, kind="ExternalInput")
    bq = nc.dram_tensor("bq", [GD, 1], f32, kind="ExternalInput")
    bk = nc.dram_tensor("bk", [GD, 1], f32, kind="ExternalInput")
    ident = nc.dram_tensor("ident", [P, P], bf16, kind="ExternalInput")
    out = nc.dram_tensor("out", [S, D], bf16, kind="ExternalOutput")

    with tile.TileContext(nc) as tc, ExitStack() as ctx:
        singles = ctx.enter_context(tc.tile_pool(name="singles", bufs=1))
        probs_pool = ctx.enter_context(tc.tile_pool(name="probs", bufs=6))
        norm_pool = ctx.enter_context(tc.tile_pool(name="normp", bufs=3))
        small = ctx.enter_context(tc.tile_pool(name="small", bufs=2))
        outs_pool = ctx.enter_context(tc.tile_pool(name="outs", bufs=6))
        psum = ctx.enter_context(tc.tile_pool(name="psum", bufs=1, space="PSUM"))

        # ---- persistent SBUF ----
        w3_sb = singles.tile([P, KC, 3 * GD], bf16)
        wo_sb = singles.tile([P, 2, D], bf16)
        bq_sb = singles.tile([P, 2, 1], f32)
        bk_sb = singles.tile([P, 2, 1], f32)
        xq_sb = singles.tile([P, KC, S], bf16)
        xk_sb = singles.tile([P, KC, S], bf16)
        xv_sb = singles.tile([P, KC, S], bf16)
        qT_sb = [singles.tile([P, S], bf16, name=f"qT{t}") for t in range(2)]
        kT_sb = [singles.tile([P, S], bf16, name=f"kT{t}") for t in range(2)]
        # v_aug: per (j-in-tile, jt, head): [v_h | 1]
        v_sb = singles.tile([P, ST, HPG * 65], bf16)
        v4 = v_sb.rearrange("p s (h c) -> p s h c", c=65)
        nc.vector.memset(v4[:, :, :, 64:65], 1.0)
        # attT, ready for out-proj: row p = hp*64+e of pair c, col = i
        att2 = singles.tile([P, 2, S], bf16)

        # ---- input DMAs, ordered by consumer deadline (DMA_ENGINES
        # transfers serialize in emission order) ----
        w3_r = wqkv.rearrange("(c p) m -> p c m", p=P)
        xq_r = xqT.rearrange("(c p) s -> p c s", p=P)
        xk_r = xkT.rearrange("(c p) s -> p c s", p=P)
        xv_r = xvT.rearrange("(c p) s -> p c s", p=P)
        QS = S // 4

        def ldx(dst, src, q0, q1):
            nc.sync.dma_start(out=dst[:, :, q0 * QS:q1 * QS],
                              in_=src[:, :, q0 * QS:q1 * QS])

        ident_sb = singles.tile([P, P], bf16)
        nc.sync.dma_start(out=w3_sb[:, :, 0:2 * GD], in_=w3_r[:, :, 0:2 * GD])
        nc.sync.dma_start(out=bq_sb, in_=bq.rearrange("(t p) o -> p t o", p=P))
        nc.sync.dma_start(out=bk_sb, in_=bk.rearrange("(t p) o -> p t o", p=P))
        nc.sync.dma_start(out=ident_sb, in_=ident[:, :])
        ldx(xq_sb, xq_r, 0, 1)
        ldx(xq_sb, xq_r, 1, 2)
        ldx(xk_sb, xk_r, 0, 1)
        nc.sync.dma_start(out=w3_sb[:, :, 2 * GD:3 * GD],
                          in_=w3_r[:, :, 2 * GD:3 * GD])
        ldx(xv_sb, xv_r, 0, 1)
        ldx(xk_sb, xk_r, 1, 2)
        ldx(xv_sb, xv_r, 1, 2)
        ldx(xk_sb, xk_r, 2, 3)
        ldx(xv_sb, xv_r, 2, 3)
        ldx(xk_sb, xk_r, 3, 4)
        ldx(xv_sb, xv_r, 3, 4)
        ldx(xq_sb, xq_r, 2, 4)
        nc.sync.dma_start(out=wo_sb, in_=wo.rearrange("(c p) n -> p c n", p=P))

        WQ, WK, WV = 0, GD, 2 * GD  # column offsets into w3_sb

        # ---- generator extras: aux-slot work in <=430ns chunks ----
        def g_qk_chain(x_sb, w_off, b_sb, dstT, t, sblk):
            """One [128, 512] column block of a q/k projection."""
            a = psum.tile([P, 512], f32, tag="aux", bufs=1, name="pqa")
            sl = slice(sblk * 512, (sblk + 1) * 512)
            for k2 in range(0, KC, 2):
                for k in (k2, k2 + 1):
                    nc.tensor.matmul(
                        out=a,
                        lhsT=w3_sb[:, k, w_off + t * P:w_off + (t + 1) * P],
                        rhs=x_sb[:, k, sl],
                        start=(k == 0), stop=(k == KC - 1))
                if k2 == KC - 2:
                    # emit the bias-add INSIDE the final chunk: consumers
                    # of dstT emitted in later units must see this write
                    # (Tile dependencies are emission-ordered)
                    nc.vector.tensor_scalar_add(out=dstT[t][:, sl], in0=a,
                                                scalar1=b_sb[:, t, :])
                yield

        def g_v_rot(pair, r):
            """V projection for head pair `pair`, s-tiles 4r..4r+3, with a
            DVE copy out after every second s-tile."""
            a = psum.tile([P, 512], f32, tag="aux", bufs=1, name="pva")
            for q in range(4):
                st = 4 * r + q
                for k4 in range(0, KC, 4):
                    for k in range(k4, k4 + 4):
                        nc.tensor.matmul(
                            out=a[:, q * P:(q + 1) * P],
                            lhsT=xv_sb[:, k, st * P:(st + 1) * P],
                            rhs=w3_sb[:, k, WV + pair * P:WV + (pair + 1) * P],
                            start=(k == 0), stop=(k == KC - 1))
                    if k4 == KC - 4 and q % 2 == 1:
                        # copy emitted INSIDE the final chunk of each 2-st
                        # group, before the yield (emission-order deps)
                        nc.vector.tensor_copy(
                            out=v4[:, st - 1:st + 1,
                                   2 * pair:2 * pair + 2, 0:64],
                            in_=a[:, (q - 1) * P:(q + 1) * P].rearrange(
                                "p (q2 h e) -> p q2 h e", q2=2, h=2))
                    yield

        def g_po(st):
            """Out-projection for s-tile st (overlaps the last block). The
            out-DMA is emitted one chunk after the copy so the SP sequencer
            never parks on a long data wait (that would delay the
            transposes queued behind it)."""
            osb = outs_pool.tile([P, D], bf16, tag="osb", name="osb")
            for nb in range(2):
                a = psum.tile([P, 512], f32, tag="aux", bufs=1, name="poa")
                for c in range(2):
                    nc.tensor.matmul(
                        out=a,
                        lhsT=att2[:, c, st * P:(st + 1) * P],
                        rhs=wo_sb[:, c, nb * 512:(nb + 1) * 512],
                        start=(c == 0), stop=(c == 1))
                    yield
                nc.vector.tensor_copy(out=osb[:, nb * 512:(nb + 1) * 512],
                                      in_=a)
                yield
            yield
            nc.sync.dma_start(out=out[st * P:(st + 1) * P, :], in_=osb)

        # ---- PE warm-up: the TensorE p-state ramps over ~3us of continuous
        # work; the input DMAs take ~6us, so burn the wait on dummy matmuls
        # (into the aux psum slot, source is a memset tile) so the first
        # projection chains dispatch at full clock ----
        wsrc = singles.tile([P, 512], bf16)
        nc.vector.memset(wsrc, 0.03)
        wps = psum.tile([P, 512], f32, tag="aux", bufs=1, name="warm")
        for i in range(52):
            nc.tensor.matmul(out=wps[:, 0:P], lhsT=wsrc[:, 0:P],
                             rhs=wsrc[:, 0:P], start=True, stop=True)
        for i in range(4):
            nc.tensor.matmul(out=wps, lhsT=wsrc[:, 0:P], rhs=wsrc,
                             start=True, stop=True)

        # ---- prologue: q0 both half-0 chains + k0 chain 0 through the sc
        # slots (k0 chain 1 rides the aux slot early in block 0) ----
        pq0 = psum.tile([P, 1024], f32, tag="sc", bufs=2, name="pq0")
        pk0 = psum.tile([P, 1024], f32, tag="sc", bufs=2, name="pk0")

        def proto_chain(x_sb, w_off, b_sb, dstT, pq, sblk):
            sl = slice(sblk * 512, (sblk + 1) * 512)
            for k in range(KC):
                nc.tensor.matmul(
                    out=pq[:, sl],
                    lhsT=w3_sb[:, k, w_off:w_off + P],
                    rhs=x_sb[:, k, sl],
                    start=(k == 0), stop=(k == KC - 1))
            nc.vector.tensor_scalar_add(out=dstT[0][:, sl], in0=pq[:, sl],
                                        scalar1=b_sb[:, 0, :])

        proto_chain(xq_sb, WQ, bq_sb, qT_sb, pq0, 0)
        proto_chain(xq_sb, WQ, bq_sb, qT_sb, pq0, 1)
        proto_chain(xk_sb, WK, bk_sb, kT_sb, pk0, 0)

        # pv accumulators: allocated ONCE; 16 units of [128, 65] packed 7/7/2
        # into 3 banks (never straddling a 2KB bank). Unit u = it*2 + hp so
        # ascending-u runs are contiguous in the normalize output layout.
        # Cross-block reuse synchronizes through subtile WAR deps.
        pv = psum.tile([P, 3, 512], f32, tag="pv", bufs=1, name="pv")

        def pv_ap(u):
            b, s2 = divmod(u, 7)
            return pv[:, b, 65 * s2:65 * s2 + 65]

        def g_transp(nrm, tpair, tih):
            """PE-transpose a finished block's normalized attn into att2
            (chunked; runs as an early extra of the following block)."""
            tpt = psum.tile([P, 8, P], bf16, tag="aux", bufs=1, name="tpt")
            for t in range(8):
                nc.tensor.transpose(
                    out=tpt[:, t, :],
                    in_=nrm[:, 2 * t:2 * t + 2, :].rearrange(
                        "p a e -> p (a e)"),
                    identity=ident_sb)
                nc.vector.tensor_copy(
                    out=att2[:, tpair, tih * 1024 + t * P:
                             tih * 1024 + (t + 1) * P],
                    in_=tpt[:, t, :])
                yield

        # ---- attention block: software-pipelined (jt, hp) units ----
        def attention(pair, ih, extras, prev_norm, pre_extras=()):
            if prev_norm is not None:
                nrm_prev, ppair, pih = prev_norm()
                extras.appendleft((1, g_transp(nrm_prev, ppair, pih)))

            units = [(jt, hp) for jt in range(ST) for hp in range(2)]
            sc_tiles = {}

            def emit_scores(n):
                jt, hp = units[n]
                sct = psum.tile([P, 1024], f32, tag="sc", bufs=2, name="sct")
                for iq in range(2):
                    nc.tensor.matmul(
                        out=sct[:, iq * 512:(iq + 1) * 512],
                        lhsT=kT_sb[pair][hp * 64:(hp + 1) * 64,
                                         jt * P:(jt + 1) * P],
                        rhs=qT_sb[pair][hp * 64:(hp + 1) * 64,
                                        ih * 1024 + iq * 512:
                                        ih * 1024 + (iq + 1) * 512],
                        start=True, stop=True)
                sc_tiles[n] = sct

            emit_scores(0)
            emit_scores(1)
            for n, (jt, hp) in enumerate(units):
                pr = probs_pool.tile([P, 1024], bf16, tag="probs", name="pr")
                nc.scalar.activation(out=pr, in_=sc_tiles.pop(n),
                                     func=Act.Exp, scale=0.125)
                if n + 2 < len(units):
                    emit_scores(n + 2)
                if n == 0:
                    for fn in pre_extras:
                        fn()
                h = 2 * pair + hp
                for it in range(8):
                    u = it * 2 + hp
                    # start=True clears has_written for the WHOLE psum bank,
                    # so only the first matmul touching each bank may carry
                    # it (units 0/8/14 lead banks 0/1/2 in emission order);
                    # every other unit's first write lands on cleared
                    # has_written bits and overwrites, later ones accumulate
                    nc.tensor.matmul(
                        out=pv_ap(u),
                        lhsT=pr[:, it * P:(it + 1) * P],
                        rhs=v_sb[:, jt, h * 65:h * 65 + 65],
                        start=(jt == 0 and u in (0, 8, 14)),
                        stop=(jt == ST - 1))
                budget = 2
                while extras and extras[0][0] <= n and budget > 0:
                    try:
                        next(extras[0][1])
                        budget -= 1
                    except StopIteration:
                        extras.popleft()

            # drain leftovers (producers must be emitted before the next
            # block's consumers)
            while extras:
                try:
                    next(extras[0][1])
                except StopIteration:
                    extras.popleft()

            def norm():
                # computes nrm = pv * (1/den); the transpose of nrm into
                # att2 is done separately with PE transposes (dma_transpose
                # writes are invisible to Tile's dependency tracking, which
                # races readers against the xbar write)
                r = small.tile([P, 16, 1], f32, tag="r", name="r")
                pvb0 = pv[:, 0, 0:455].rearrange("p (s c) -> p s c", c=65)
                pvb1 = pv[:, 1, 0:455].rearrange("p (s c) -> p s c", c=65)
                pvb2 = pv[:, 2, 0:130].rearrange("p (s c) -> p s c", c=65)
                nc.vector.reciprocal(out=r[:, 0:7, 0], in_=pvb0[:, :, 64])
                nc.vector.reciprocal(out=r[:, 7:14, 0], in_=pvb1[:, :, 64])
                nc.vector.reciprocal(out=r[:, 14:16, 0], in_=pvb2[:, :, 64])
                nrm = norm_pool.tile([P, 16, 64], bf16, tag="norm",
                                     name="nrm")

                def bmul(dst, in0, rsl):
                    a1, a2 = bass.broadcast_tensor_aps(in0, rsl)
                    nc.vector.tensor_tensor(out=dst, in0=a1, in1=a2,
                                            op=Alu.mult)

                bmul(nrm[:, 0:7, :], pvb0[:, :, 0:64], r[:, 0:7, :])
                bmul(nrm[:, 7:8, :], pvb1[:, 0:1, 0:64], r[:, 7:8, :])
                bmul(nrm[:, 8:14, :], pvb1[:, 1:7, 0:64], r[:, 8:14, :])
                bmul(nrm[:, 14:16, :], pvb2[:, :, 0:64], r[:, 14:16, :])
                return nrm, pair, ih

            return norm

        # extras: (earliest_unit, generator), ordered by DMA arrival and
        # consumer deadline; at most 2 chunks advanced per unit.
        ex0 = deque([
            (4, g_qk_chain(xk_sb, WK, bk_sb, kT_sb, 0, 1)),
            (6, g_v_rot(0, 1)),
            (10, g_qk_chain(xk_sb, WK, bk_sb, kT_sb, 0, 2)),
            (12, g_v_rot(0, 2)),
            (16, g_qk_chain(xk_sb, WK, bk_sb, kT_sb, 0, 3)),
            (18, g_v_rot(0, 3)),
            (24, g_qk_chain(xq_sb, WQ, bq_sb, qT_sb, 0, 2)),
            (27, g_qk_chain(xq_sb, WQ, bq_sb, qT_sb, 0, 3)),
        ])
        ex1 = deque([
            (0, g_qk_chain(xq_sb, WQ, bq_sb, qT_sb, 1, 0)),
            (2, g_qk_chain(xk_sb, WK, bk_sb, kT_sb, 1, 0)),
            (4, g_qk_chain(xq_sb, WQ, bq_sb, qT_sb, 1, 1)),
            (6, g_v_rot(1, 0)),
            (9, g_qk_chain(xk_sb, WK, bk_sb, kT_sb, 1, 1)),
            (12, g_v_rot(1, 1)),
            (15, g_qk_chain(xk_sb, WK, bk_sb, kT_sb, 1, 2)),
            (18, g_qk_chain(xk_sb, WK, bk_sb, kT_sb, 1, 3)),
        ])
        ex2 = deque([
            (0, g_v_rot(1, 2)),
            (4, g_v_rot(1, 3)),
            (8, g_qk_chain(xq_sb, WQ, bq_sb, qT_sb, 1, 2)),
            (12, g_qk_chain(xq_sb, WQ, bq_sb, qT_sb, 1, 3)),
        ])
        ex3 = deque([(2 + 3 * i, g_po(st)) for i, st in enumerate(range(6))])

        def v00():
            for _ in g_v_rot(0, 0):
                pass

        po_rest = []
        if SERIAL:
            # bisection aid: drain every generator up front, no interleaving
            v00()
            for _, g in [*ex0, *ex1, *ex2]:
                for _ in g:
                    pass
            po_rest = [g for _, g in ex3]
            ex0, ex1, ex2, ex3 = deque(), deque(), deque(), deque()
        pending = attention(0, 0, ex0, None,
                            pre_extras=([] if SERIAL else [v00]))
        pending = attention(0, 1, ex1, pending)
        if DEBUG:
            dbg_early = nc.dram_tensor("dbg_early", [P, 1024], bf16,
                                       kind="ExternalOutput")
            nc.sync.dma_start(out=dbg_early[:, :], in_=att2[:, 0, 0:1024])
        pending = attention(1, 0, ex2, pending)
        pending = attention(1, 1, ex3, pending)
        for g in po_rest:
            for _ in g:
                pass
        nrm_last, _, _ = pending()
        if DEBUG:
            dbg_pv = nc.dram_tensor("dbg_pv", [P, 3 * 512], f32,
                                    kind="ExternalOutput")
            pvstage = singles.tile([P, 3, 512], f32)
            nc.vector.tensor_copy(out=pvstage, in_=pv)
            nc.sync.dma_start(out=dbg_pv[:, :],
                              in_=pvstage.rearrange("p a b -> p (a b)"))

        # ---- tail: out-projection for s-tiles 6..15 through a 7-slot PSUM
        # pipeline over the now-idle sc/pv banks, with the final block's
        # att2 assembly done by PE transposes into the aux bank (no
        # DMA-queue slot guards; PE never idles, so it keeps its full
        # p-state). s-tiles 6,7 need only the ih0 half of att2, so their
        # matmuls run DURING the final normalize; s-tile 8+t consumes
        # exactly transpose t, so everything pipelines. Copies alternate
        # ScalarE/DVE; out-DMAs are emitted late so the SP sequencer never
        # parks on a data wait.
        scA = psum.tile([P, 1024], f32, tag="sc", bufs=2, name="tscA")
        scB = psum.tile([P, 1024], f32, tag="sc", bufs=2, name="tscB")
        auxT = psum.tile([P, 8, P], bf16, tag="aux", bufs=1, name="taux")
        slots = [scA[:, 0:512], scA[:, 512:1024],
                 scB[:, 0:512], scB[:, 512:1024],
                 pv[:, 0, 0:512], pv[:, 1, 0:512], pv[:, 2, 0:512]]
        osbs = {}
        dma_queue = []

        def po_group(i, st, nb):
            slot = slots[i % 7]
            for c in range(2):
                nc.tensor.matmul(
                    out=slot,
                    lhsT=att2[:, c, st * P:(st + 1) * P],
                    rhs=wo_sb[:, c, nb * 512:(nb + 1) * 512],
                    start=(c == 0), stop=(c == 1))
            if nb == 0:
                osbs[st] = outs_pool.tile([P, D], bf16, tag="osb",
                                          name="osbt")
            o = osbs[st]
            # s-tiles 6/7 copy on ScalarE (it idles right after the last
            # exp, while DVE is busy with the final normalize)
            if st < 8 or st % 2 == 0:
                nc.scalar.copy(out=o[:, nb * 512:(nb + 1) * 512], in_=slot)
            else:
                nc.vector.tensor_copy(out=o[:, nb * 512:(nb + 1) * 512],
                                      in_=slot)
            if nb == 1:
                dma_queue.append(st)
                if len(dma_queue) > 1:
                    st_d = dma_queue.pop(0)
                    nc.sync.dma_start(out=out[st_d * P:(st_d + 1) * P, :],
                                      in_=osbs.pop(st_d))

        gi = 0
        for st in (6, 7):
            for nb in range(2):
                po_group(gi, st, nb)
                gi += 1
        # PE transposes of the final block's normalized attn: t covers
        # i-tile 8+t; copies chase through the 4 aux sub-slots
        for t in range(8):
            tp = auxT[:, t, :]
            nc.tensor.transpose(
                out=tp,
                in_=nrm_last[:, 2 * t:2 * t + 2, :].rearrange(
                    "p a e -> p (a e)"),
                identity=ident_sb)
            dst = att2[:, 1, 1024 + t * P:1024 + (t + 1) * P]
            if t % 2 == 0:
                nc.scalar.copy(out=dst, in_=tp)
            else:
                nc.vector.tensor_copy(out=dst, in_=tp)
        for st in range(8, 16):
            for nb in range(2):
                po_group(gi, st, nb)
                gi += 1
        for st_d in dma_queue:
            nc.sync.dma_start(out=out[st_d * P:(st_d + 1) * P, :],
                              in_=osbs.pop(st_d))

        if DEBUG:
            dbg_att2 = nc.dram_tensor("dbg_att2", [P, 2 * S], bf16,
                                      kind="ExternalOutput")
            dbg_qk = nc.dram_tensor("dbg_qk", [P, 4 * S], bf16,
                                    kind="ExternalOutput")
            dbg_v = nc.dram_tensor("dbg_v", [P, ST * HPG * 65], bf16,
                                   kind="ExternalOutput")
            nc.sync.dma_start(out=dbg_att2[:, :],
                              in_=att2.rearrange("p a b -> p (a b)"))
            nc.sync.dma_start(out=dbg_qk[:, 0:S], in_=qT_sb[0])
            nc.sync.dma_start(out=dbg_qk[:, S:2 * S], in_=qT_sb[1])
            nc.sync.dma_start(out=dbg_qk[:, 2 * S:3 * S], in_=kT_sb[0])
            nc.sync.dma_start(out=dbg_qk[:, 3 * S:4 * S], in_=kT_sb[1])
            nc.sync.dma_start(out=dbg_v[:, :],
                              in_=v_sb.rearrange("p a b -> p (a b)"))

    nc.finalize()
    return nc


def kernel(Q, K, V, Wq, bq, Wk, bk, Wv, bv, Wo, bo):
    from concourse.bass_utils import run_bass_kernel_spmd

    f32 = np.float32
    Q = np.asarray(Q, f32)
    K = np.asarray(K, f32)
    V = np.asarray(V, f32)
    Wq = np.asarray(Wq, f32)
    Wk = np.asarray(Wk, f32)
    Wv = np.asarray(Wv, f32)
    Wo = np.asarray(Wo, f32)
    bq = np.asarray(bq, f32)
    bk = np.asarray(bk, f32)
    bv = np.asarray(bv, f32)
    bo = np.asarray(bo, f32)

    xT = {}
    for b in range(B):
        xT[('q', b)] = np.ascontiguousarray(Q[b].T).astype(BF16)
        xT[('k', b)] = np.ascontiguousarray(K[b].T).astype(BF16)
        xT[('v', b)] = np.ascontiguousarray(V[b].T).astype(BF16)

    in_maps = []
    for c in range(NCORES):
        b, g = c // GROUPS, c % GROUPS
        sl = slice(g * GD, (g + 1) * GD)
        wqkv = np.concatenate([Wq[:, sl], Wk[:, sl], Wv[:, sl]], axis=1)
        in_maps.append({
            "xqT": xT[('q', b)],
            "xkT": xT[('k', b)],
            "xvT": xT[('v', b)],
            "wqkv": np.ascontiguousarray(wqkv).astype(BF16),
            "wo": np.ascontiguousarray(Wo[sl, :]).astype(BF16),
            "bq": np.ascontiguousarray(bq[sl].reshape(GD, 1)),
            "bk": np.ascontiguousarray(bk[sl].reshape(GD, 1)),
            "ident": np.eye(P, dtype=BF16),
        })

    if "nc" not in _cached:
        _cached["nc"] = _build_bass()
    nc = _cached["nc"]

    try:
        res = run_bass_kernel_spmd(nc, in_maps, core_ids=list(range(NCORES)))
    except ModuleNotFoundError:
        # BASS_TRACE set but the axon ntff hook isn't shipped in this
        # container - retry untraced
        os.environ["BASS_NEVER_TRACE"] = "1"
        res = run_bass_kernel_spmd(nc, in_maps, core_ids=list(range(NCORES)))
    if res.exec_time_ns is not None:
        print(f"HW exec time: {res.exec_time_ns} ns")

    bo_eff = (bv @ Wo + bo).astype(f32)
    out = np.zeros((B, S, D), f32)
    for c in range(NCORES):
        b = c // GROUPS
        out[b] += np.asarray(res.results[c]["out"]).astype(f32)
    out += bo_eff
    return out
